# revision 1
# baseline (speedup 1.0000x reference)
"""2-layer GraphSAGE (mean aggr + BN(eval) + ReLU) on Trainium2, 8-core SPMD.

Strategy (graph/data parallel, dst-node sharding, host-mediated all-to-all):
  - Host: relabel nodes by in-degree (descending), deal 128-node chunks
    round-robin to the 8 cores so chunk ci holds same-degree nodes on every
    core (shared per-chunk pad depth K[ci], SPMD). The host performs the
    all-to-all exchange of source features: for each core it stages the
    edge-gathered source-feature slabs expT[ch, slot] (bf16, channel-major,
    slot = (chunk, k, dst-lane), zero-padded to K[ci] in-edges per node).
  - Device layer (identical structure for both layers):
      expT streams into SBUF in 8 big pipelined section DMAs (it stays
      resident: 154KB/partition). Per chunk:
        ps_A = sum_k slab_k^T @ Wproj      (= agg^T @ Wproj, K matmuls
                                            accumulated in PSUM)
        ps_B = own^T @ Wself + ones^T @ brow   (bias via K=1 matmul)
        out  = ps_A * invdeg[dst] + ps_B   (DVE scalar_tensor_tensor,
                                            invdeg fp32 per-partition)
      (+ ReLU for layer 1). Outputs collect in SBUF lane-major and are
      written once at the end ([128, CPC*chout], host unshuffles).
  - Between launches the host assembles h, re-runs the same index map to
    stage layer 2's slabs (all-to-all of h), and unpermutes the final out.
"""

import numpy as np

import concourse.bacc as bacc
import concourse.mybir as mybir
import concourse.tile as tile
from concourse.bass_utils import run_bass_kernel_spmd

F32 = mybir.dt.float32
BF16 = mybir.dt.bfloat16
OP = mybir.AluOpType
BF16_NP = mybir.dt.np(mybir.dt.bfloat16)

N_CORES = 8
P = 128

N_NODES = 50000
NP_PAD = 50176            # 392 chunks of 128
E = 600000
C_IN, C_HID, C_OUT = 128, 128, 64
CPC = NP_PAD // P // N_CORES   # 49 chunks per core
NPC = CPC * P                  # 6272 nodes per core
BN_EPS = 1e-5
NSEC = 16                      # expT section loads


def _preprocess(edge_index):
    """Degree-sort relabeling + slot map for the edge-gathered slabs."""
    src = np.asarray(edge_index[0]).astype(np.int64)
    dst = np.asarray(edge_index[1]).astype(np.int64)
    ne = src.shape[0]
    deg = np.bincount(dst, minlength=NP_PAD).astype(np.int64)

    nodeorder = np.argsort(-deg, kind="stable")        # rank -> node
    rank = np.empty(NP_PAD, np.int64)
    rank[nodeorder] = np.arange(NP_PAD)

    gdeg = deg[nodeorder].reshape(NP_PAD // P, P)
    K = np.maximum(gdeg.reshape(CPC, N_CORES, P).max(axis=(1, 2)), 1)
    colstart = np.zeros(CPC, np.int64)
    colstart[1:] = np.cumsum(K)[:-1]
    S_total = int(K.sum())

    key = rank[dst]
    order = np.argsort(key, kind="stable")
    r_s = key[order]
    src_s = src[order].astype(np.int32)
    starts = np.searchsorted(r_s, r_s, side="left")
    k_in = np.arange(ne) - starts
    g = r_s // P
    core = g % N_CORES
    ci = g // N_CORES
    p = r_s % P
    J = (colstart[ci] + k_in) * P + p
    slot_src = []
    for c in range(N_CORES):
        m = core == c
        a = np.full(S_total * P, -1, np.int32)
        a[J[m]] = src_s[m]
        slot_src.append(a)

    node_of = []
    ivd_t = (1.0 / np.maximum(deg, 1.0)).astype(np.float32)
    slot_scale = []
    for c in range(N_CORES):
        idx = (np.arange(CPC)[:, None] * N_CORES + c) * P + np.arange(P)[None, :]
        nodes = nodeorder[idx]                         # [CPC, P]
        node_of.append(nodes.reshape(-1).astype(np.int32))
        iv = ivd_t[nodes]                              # [CPC, P]
        sc = np.concatenate(
            [np.tile(iv[ci], int(K[ci])) for ci in range(CPC)])
        slot_scale.append(sc.astype(np.float32))       # [S_total*P]
    return K, slot_src, node_of, slot_scale


def _mk_nc():
    return bacc.Bacc(
        "TRN2",
        target_bir_lowering=False,
        debug=False,
        enable_asserts=False,
        num_devices=N_CORES,
    )


def build_layer(K, chout, relu, out_bf16):
    """One GraphSAGE layer. expT slabs (pre-scaled by invdeg) + own + W -> out."""
    S_total = int(K.sum())
    csum = np.zeros(CPC + 1, np.int64)
    csum[1:] = np.cumsum(K)
    # section boundaries (chunk indices): progressive sizes — small first so
    # compute starts early, growing as the pipeline fills
    fracs = np.cumsum([0, 1.5, 1.5, 2, 3, 5, 6, 7, 8, 8.5,
                       9, 9, 9, 9, 9, 9, 3.5])
    fracs = fracs / fracs[-1]
    bounds = [0]
    for s in range(1, NSEC):
        b = int(np.searchsorted(csum, S_total * fracs[s]))
        bounds.append(max(b, bounds[-1]))
    bounds.append(CPC)

    nc = _mk_nc()
    d_exp = nc.dram_tensor("expT", (P, S_total * P), BF16, kind="ExternalInput")
    d_own = nc.dram_tensor("ownT", (P, NPC), BF16, kind="ExternalInput")
    d_wa = nc.dram_tensor("wa", (C_IN, chout), BF16, kind="ExternalInput")
    d_wb = nc.dram_tensor("wb", (C_IN, chout), BF16, kind="ExternalInput")
    d_brow = nc.dram_tensor("brow", (1, chout), BF16, kind="ExternalInput")
    d_ones = nc.dram_tensor("ones", (1, P), BF16, kind="ExternalInput")
    out_dt = BF16 if out_bf16 else F32
    d_out = nc.dram_tensor("out", (P, CPC * chout), out_dt, kind="ExternalOutput")
    AF = mybir.ActivationFunctionType

    with tile.TileContext(nc) as tc:
        with (
            tc.tile_pool(name="const", bufs=1) as cp,
            tc.tile_pool(name="psA", bufs=4, space="PSUM") as pA,
            tc.tile_pool(name="psW", bufs=1, space="PSUM") as pW,
        ):
            def cload(name, d, shape, dt=BF16):
                # scalar-engine HWDGE ring: parallel to the big section loads
                t = cp.tile(shape, dt, tag=name)
                nc.scalar.dma_start(t[:], d.ap()[:, :])
                return t

            # wa rides the sync ring ahead of the sections: it gates the
            # first real matmuls and is tiny
            t_wa = cp.tile([C_IN, chout], BF16, tag="wa")
            nc.sync.dma_start(t_wa[:], d_wa.ap()[:, :])
            t_exp = cp.tile([P, S_total * P], BF16, tag="exp")
            for s in range(NSEC):
                a = int(csum[bounds[s]]) * P
                b = int(csum[bounds[s + 1]]) * P
                if b > a:
                    nc.sync.dma_start(t_exp[:, a:b], d_exp.ap()[:, a:b])
            t_own = cload("own", d_own, [P, NPC])
            t_wb = cload("wb", d_wb, [C_IN, chout])
            t_brow = cload("brow", d_brow, [1, chout])
            t_ones = cload("ones", d_ones, [1, P])
            t_hall = cp.tile([P, CPC * chout], out_dt, tag="hall")

            # HAM warmup: keep the PE busy while the first sections stream
            # in. The operand comes from a memset (no DMA dependency), so the
            # warmup starts right after the preamble.
            t_warm = cp.tile([P, P], BF16, tag="warm")
            nc.vector.memset(t_warm[:], 1.0)
            ps_w = pW.tile([P, chout], F32)
            for w in range(40):
                nc.tensor.matmul(out=ps_w[:], lhsT=t_warm[:],
                                 rhs=t_warm[:, :chout],
                                 start=(w == 0), stop=(w == 39))

            for ci in range(CPC):
                k = int(K[ci])
                c0 = int(csum[ci])
                ps = pA.tile([P, chout], F32)
                for kk in range(k):
                    nc.tensor.matmul(
                        out=ps[:],
                        lhsT=t_exp[:, (c0 + kk) * P:(c0 + kk + 1) * P],
                        rhs=t_wa[:],
                        start=(kk == 0),
                        stop=False,
                    )
                nc.tensor.matmul(out=ps[:],
                                 lhsT=t_own[:, ci * P:(ci + 1) * P],
                                 rhs=t_wb[:], start=False, stop=False)
                nc.tensor.matmul(out=ps[:], lhsT=t_ones[:], rhs=t_brow[:],
                                 start=False, stop=True)
                dst_sl = t_hall[:, ci * chout:(ci + 1) * chout]
                nc.scalar.activation(out=dst_sl, in_=ps[:],
                                     func=AF.Relu if relu else AF.Identity)
                # flush finished output stripes while compute continues
                if ci in (15, 31, 43, CPC - 1):
                    prev = {15: 0, 31: 16, 43: 32, CPC - 1: 44}[ci]
                    nc.scalar.dma_start(
                        d_out.ap()[:, prev * chout:(ci + 1) * chout],
                        t_hall[:, prev * chout:(ci + 1) * chout])

    nc.compile()
    return nc


_cache = {}


def _get_programs(K):
    key = tuple(int(x) for x in K)
    if key not in _cache:
        _cache[key] = (
            build_layer(K, C_HID, relu=True, out_bf16=True),
            build_layer(K, C_OUT, relu=False, out_bf16=False),
        )
    return _cache[key]


def _expand(tabT_ext, slot_idx, scale):
    """tabT_ext f32 [128, NP_PAD+1] (last col zero), slot_idx int32 (-1 pad),
    scale f32 per slot column; single rounding to bf16."""
    idx = np.where(slot_idx < 0, NP_PAD, slot_idx)
    e = tabT_ext[:, idx] * scale[None, :]
    return np.ascontiguousarray(e.astype(BF16_NP))


def _unshuffle(part, chout):
    """[P, CPC*chout] lane-major -> [NPC, chout] row-major."""
    return np.ascontiguousarray(
        part.reshape(P, CPC, chout).transpose(1, 0, 2).reshape(NPC, chout))


def kernel(x, edge_index, W1_l, W1_r, b1, bn_gamma, bn_beta, bn_mean, bn_var,
           W2_l, W2_r, b2, _results=None):
    K, slot_src, node_of, slot_scale = _preprocess(edge_index)
    nc1, nc2 = _get_programs(K)

    # BN folding (float64 for accuracy): h = gamma*(z - mean)/sqrt(var+eps)+beta
    s = (np.asarray(bn_gamma, np.float64)
         / np.sqrt(np.asarray(bn_var, np.float64) + BN_EPS))
    w1l_f = (np.asarray(W1_l, np.float64) * s[None, :]).astype(BF16_NP)
    w1r_f = (np.asarray(W1_r, np.float64) * s[None, :]).astype(BF16_NP)
    c1 = ((np.asarray(b1, np.float64) - np.asarray(bn_mean, np.float64)) * s
          + np.asarray(bn_beta, np.float64)).astype(BF16_NP).reshape(1, C_HID)
    w2l = np.asarray(W2_l, np.float32).astype(BF16_NP)
    w2r = np.asarray(W2_r, np.float32).astype(BF16_NP)
    b2r = np.asarray(b2, np.float32).astype(BF16_NP).reshape(1, C_OUT)
    ones = np.ones((1, P), np.float32).astype(BF16_NP)

    x_pad = np.zeros((NP_PAD + 1, C_IN), np.float32)
    x_pad[:N_NODES] = np.asarray(x, np.float32)
    # round features once to bf16, then expand/scale from the f32 view of that
    xT_bf = np.ascontiguousarray(x_pad.astype(BF16_NP).T)   # [128, NP+1]
    xT_f = xT_bf.astype(np.float32)

    maps1 = []
    for c in range(N_CORES):
        maps1.append(dict(
            expT=_expand(xT_f, slot_src[c], slot_scale[c]),
            ownT=np.ascontiguousarray(xT_bf[:, node_of[c]]),
            wa=w1l_f, wb=w1r_f, brow=c1, ones=ones,
        ))
    r1 = run_bass_kernel_spmd(nc1, maps1, list(range(N_CORES)))

    # assemble h (original node ids), then all-to-all for layer 2
    hT_bf = np.zeros((C_HID, NP_PAD + 1), BF16_NP)
    for c in range(N_CORES):
        h_part = _unshuffle(np.asarray(r1.results[c]["out"]), C_HID)
        hT_bf[:, node_of[c]] = h_part.T
    hT_bf[:, NP_PAD] = 0
    hT_f = hT_bf.astype(np.float32)

    maps2 = []
    for c in range(N_CORES):
        maps2.append(dict(
            expT=_expand(hT_f, slot_src[c], slot_scale[c]),
            ownT=np.ascontiguousarray(hT_bf[:, node_of[c]]),
            wa=w2l, wb=w2r, brow=b2r, ones=ones,
        ))
    r2 = run_bass_kernel_spmd(nc2, maps2, list(range(N_CORES)))

    out = np.zeros((NP_PAD, C_OUT), np.float32)
    for c in range(N_CORES):
        out[node_of[c]] = _unshuffle(np.asarray(r2.results[c]["out"]), C_OUT)
    if _results is not None:
        _results.extend([r1, r2])
    return np.ascontiguousarray(out[:N_NODES])



# revision 19
# speedup vs baseline: 1.0635x; 1.0635x over previous
"""2-layer GraphSAGE (mean aggr + BN(eval) + ReLU) on Trainium2, 8-core SPMD.

Strategy (dst-node sharding, host-mediated all-to-all, fp8 slabs):
  - Host: relabel nodes by in-degree (desc), deal 128-node chunks round-robin
    to 8 cores (chunk ci has ~equal degrees on every core -> shared pad depth
    K[ci], SPMD). Host stages the edge-gathered source features as fp8-e3m4
    slabs pre-scaled by invdeg * 2^s(ci) (per-chunk pow2 keeps values in
    e3m4's sweet range; the inverse pow2 is applied by ACT at psum readout).
  - Layer 1 (launch 1): W1 matrices ride STATIONARY in the PE (no per-matmul
    LDWEIGHTS), slabs stream as the moving operand. High-K chunks are
    pre-reduced on the Vector engine (fold-in-half tree, f32 scratch, bf16
    final) so PE and DVE share the aggregation work. Per chunk:
       psum[chout,dst] = sum_k W1l^T slab_k  (or W1l^T dve_sum)
                       + W1r^T own           (own = x*2^s, bf16)
       h = ACT(Relu, scale=2^-s, bias=c1)    (BN folded into W1/c1)
       psum2[128,dst] = [W2l|W2r]^T h        (fused projection)
       y2  = ACT(Identity, bias=[0;b2])      -> [y2l;y2r] bf16 out
    Only y2 (128 rows: 64 y2l + 64 y2r) returns to the host - h never does.
  - Host: all-to-all/regather of y2l into layer-2 slabs: fp8 stacked PAIRS
    ([2x64ch, dst]) pre-scaled by invdeg * 2^s2(ci); y2r (own dst, includes
    b2) stays bf16, pre-scaled by 2^s2(ci).
  - Layer 2 (launch 2): aggregation is a pure sum: stacked pairs contract
    with a constant [I64;I64] stationary (psum += slab_a + slab_b); DVE
    pre-folds the high-K chunks; y2r joins via an I64 matmul; ACT scales by
    2^-s2 to f32 out. No weights needed on device at all.
"""

import numpy as np

import concourse.bacc as bacc
import concourse.mybir as mybir
import concourse.tile as tile
from concourse.bass_utils import run_bass_kernel_spmd

F32 = mybir.dt.float32
BF16 = mybir.dt.bfloat16
E3 = mybir.dt.float8e3
OP = mybir.AluOpType
AF = mybir.ActivationFunctionType
BF16_NP = mybir.dt.np(mybir.dt.bfloat16)
E3_NP = mybir.dt.np(mybir.dt.float8e3)

N_CORES = 8
P = 128
HP = 64

N_NODES = 50000
NP_PAD = 50176            # 392 chunks of 128
E = 600000
C_IN, C_HID, C_OUT = 128, 128, 64
CPC = NP_PAD // P // N_CORES   # 49 chunks per core
NPC = CPC * P                  # 6272 nodes per core
BN_EPS = 1e-5

# tuning knobs
DVE_SLOT_BUDGET_L1 = 190       # ~ slots pre-reduced on DVE in layer 1
DVE_PAIR_BUDGET_L2 = 60        # ~ pair-columns pre-reduced on DVE in layer 2
NSEC1 = 16
NSEC2 = 10


def _fold_schedule(m):
    """Fold-in-half schedule for m columns -> 1.

    Returns list of ops on a scratch holding the current level:
      ('L0', h)          : scratch[0:h] = in[0:h] + in[h:2h]   (m even, h=m//2)
      ('odd', mcur)      : scratch[0] += scratch[mcur-1]
      ('fold', h)        : scratch[0:h] += scratch[h:2h]
      ('final',)         : out = scratch[0] + scratch[1]  (bf16)
    Caller handles m==1 (no DVE) and m==2 ('L0final') specially.
    """
    assert m % 2 == 0 and m >= 4
    ops = [("L0", m // 2)]
    m = m // 2
    while m > 2:
        if m % 2 == 1:
            ops.append(("odd", m))
            m -= 1
        if m == 2:
            break
        ops.append(("fold", m // 2))
        m = m // 2
    ops.append(("final",))
    return ops


def _preprocess(edge_index, xmax):
    """Degree-sort relabeling + slot maps + per-chunk pow2 scales.

    xmax: [NP_PAD] inf-norm of each node's feature row (for clip-free scales).
    """
    src = np.asarray(edge_index[0]).astype(np.int64)
    dst = np.asarray(edge_index[1]).astype(np.int64)
    ne = src.shape[0]
    deg = np.bincount(dst, minlength=NP_PAD).astype(np.int64)

    nodeorder = np.argsort(-deg, kind="stable")        # rank -> node
    rank = np.empty(NP_PAD, np.int64)
    rank[nodeorder] = np.arange(NP_PAD)

    gdeg = deg[nodeorder].reshape(NP_PAD // P, P)
    gdeg3 = gdeg.reshape(CPC, N_CORES, P)
    K = np.maximum(gdeg3.max(axis=(1, 2)), 1)
    degmed = np.maximum(np.median(gdeg3.reshape(CPC, -1), axis=1), 1.0)
    s1 = 2.0 ** np.round(np.log2(2.0 * degmed))        # per-chunk pow2
    # cap: no slab value may exceed e3m4 range after scaling
    ci_of_all = rank[dst] // P // N_CORES
    ivd_e_all = 1.0 / np.maximum(deg[dst], 1.0)
    mx1 = np.zeros(CPC)
    np.maximum.at(mx1, ci_of_all, np.asarray(xmax)[src] * ivd_e_all)
    for ci in range(CPC):
        while mx1[ci] * s1[ci] > 14.0:
            s1[ci] /= 2.0

    # DVE chunk sets: assign largest-K chunks until the slot budget is used
    dve1 = set()
    tot = 0
    for ci in range(CPC):
        if tot + K[ci] <= DVE_SLOT_BUDGET_L1 and K[ci] >= 4:
            dve1.add(ci)
            tot += int(K[ci])
    K1 = np.array([int(k) + (int(k) % 2 if ci in dve1 else 0)
                   for ci, k in enumerate(K)])         # DVE chunks: even K

    K2p = (K + 1) // 2                                  # layer-2 pair cols
    dve2 = set()
    tot = 0
    for ci in range(CPC):
        if tot + K2p[ci] <= DVE_PAIR_BUDGET_L2 and K2p[ci] >= 4:
            dve2.add(ci)
            tot += int(K2p[ci])
    K2 = np.array([int(k) + (int(k) % 2 if ci in dve2 else 0)
                   for ci, k in enumerate(K2p)])

    # storage/processing order: interleave DVE chunks among PE chunks
    dlist = [ci for ci in range(CPC) if ci in dve1]
    plist = [ci for ci in range(CPC) if ci not in dve1]
    order = []
    di = pi = 0
    ratio = max(len(plist) / max(len(dlist), 1), 1.0)
    while di < len(dlist) or pi < len(plist):
        if di < len(dlist) and (pi >= len(plist) or pi >= ratio * di):
            order.append(dlist[di]); di += 1
        else:
            order.append(plist[pi]); pi += 1
    order = np.array(order)

    csum1 = np.zeros(CPC + 1, np.int64)
    csum1[1:] = np.cumsum(K1[order])
    csum2 = np.zeros(CPC + 1, np.int64)
    csum2[1:] = np.cumsum(K2[order])

    # edge -> (core, chunk, k, lane)
    key = rank[dst]
    eorder = np.argsort(key, kind="stable")
    r_s = key[eorder]
    src_s = src[eorder].astype(np.int64)
    starts = np.searchsorted(r_s, r_s, side="left")
    k_in = np.arange(ne) - starts
    g = r_s // P
    core = g % N_CORES
    ci_of_e = g // N_CORES
    lane = r_s % P
    pos_of = np.empty(CPC, np.int64)                    # chunk -> storage pos
    pos_of[order] = np.arange(CPC)

    ivd = (1.0 / np.maximum(deg, 1.0)).astype(np.float64)

    # L1 slot -> src node / scale (per core)
    S1 = int(csum1[-1])
    J1 = (csum1[pos_of[ci_of_e]] + k_in) * P + lane
    # L2: stacked pairs: pair p, half = k%2; column = (csum2+p)*P+lane,
    # row-half = half
    S2 = int(csum2[-1])
    J2 = (csum2[pos_of[ci_of_e]] + k_in // 2) * P + lane
    half2 = (k_in % 2).astype(np.int64)

    slot1_src, slot1_sc = [], []
    slot2_src, slot2_sc = [], []
    node_of = []
    for c in range(N_CORES):
        m = core == c
        a = np.full(S1 * P, -1, np.int64)
        a[J1[m]] = src_s[m]
        slot1_src.append(a)
        sc = np.zeros(S1 * P, np.float32)
        sc[J1[m]] = (ivd[dst[eorder]][m] * s1[ci_of_e[m]]).astype(np.float32)
        slot1_sc.append(sc)

        at = np.full(S2 * P, -1, np.int64)
        ab = np.full(S2 * P, -1, np.int64)
        mt = m & (half2 == 0)
        mb = m & (half2 == 1)
        at[J2[mt]] = src_s[mt]
        ab[J2[mb]] = src_s[mb]
        slot2_src.append((at, ab))
        st = np.zeros(S2 * P, np.float32)
        sb = np.zeros(S2 * P, np.float32)
        st[J2[mt]] = ivd[dst[eorder]][mt].astype(np.float32)
        sb[J2[mb]] = ivd[dst[eorder]][mb].astype(np.float32)
        slot2_sc.append((st, sb))

        idx = (order[:, None] * N_CORES + c) * P + np.arange(P)[None, :]
        node_of.append(nodeorder[idx.reshape(-1)].astype(np.int64))

    return dict(K1=K1, K2=K2, order=order, csum1=csum1, csum2=csum2,
                dve1=dve1, dve2=dve2, s1=s1, degmed=degmed,
                edge_ci=ci_of_all, edge_ivd=ivd_e_all, edge_src=src,
                slot1_src=slot1_src,
                slot1_sc=slot1_sc, slot2_src=slot2_src, slot2_sc=slot2_sc,
                node_of=node_of, S1=S1, S2=S2)


def _mk_nc():
    return bacc.Bacc(
        "TRN2",
        target_bir_lowering=False,
        debug=False,
        enable_asserts=False,
        num_devices=N_CORES,
    )


def _sections(csum, nsec, order_len):
    """Progressive chunk-boundary sections over the slab stream."""
    S = int(csum[-1])
    fracs = np.cumsum([0] + [1.5, 1.5, 2, 3, 5] + [7] * (nsec - 6) + [4])
    fracs = fracs / fracs[-1]
    bounds = [0]
    for s in range(1, nsec):
        b = int(np.searchsorted(csum, S * fracs[s]))
        bounds.append(min(max(b, bounds[-1]), order_len))
    bounds.append(order_len)
    return bounds


def build_layer1(pp):
    K1 = pp["K1"]; order = pp["order"]; csum = pp["csum1"]
    dve1 = pp["dve1"]; s1 = pp["s1"]
    S1 = pp["S1"]
    scrw = max((int(K1[ci]) // 2 for ci in dve1), default=1)
    bounds = _sections(csum, NSEC1, CPC)

    nc = _mk_nc()
    d_exp = nc.dram_tensor("expT", (P, S1 * P), E3, kind="ExternalInput")
    d_own = nc.dram_tensor("ownT", (P, NPC), BF16, kind="ExternalInput")
    d_w1 = nc.dram_tensor("w1", (C_IN, 2 * C_HID), BF16, kind="ExternalInput")
    d_w2 = nc.dram_tensor("w2", (C_HID, P), BF16, kind="ExternalInput")
    d_c1 = nc.dram_tensor("c1", (P, 1), F32, kind="ExternalInput")
    d_b2 = nc.dram_tensor("b2", (P, 1), F32, kind="ExternalInput")
    d_y2 = nc.dram_tensor("y2", (P, CPC * P), BF16, kind="ExternalOutput")

    with tile.TileContext(nc) as tc:
        with (
            tc.tile_pool(name="const", bufs=1) as cp,
            tc.tile_pool(name="dsum", bufs=4) as dp,
            tc.tile_pool(name="scr", bufs=2) as sp,
            tc.tile_pool(name="psA", bufs=4, space="PSUM") as pA,
            tc.tile_pool(name="psP", bufs=2, space="PSUM") as pP,
            tc.tile_pool(name="psW", bufs=1, space="PSUM") as pW,
        ):
            # small consts on the scalar HWDGE ring (parallel to sections)
            t_w1 = cp.tile([C_IN, 2 * C_HID], BF16, tag="w1")
            nc.scalar.dma_start(t_w1[:], d_w1.ap()[:, :])
            t_w2 = cp.tile([C_HID, P], BF16, tag="w2")
            nc.scalar.dma_start(t_w2[:], d_w2.ap()[:, :])
            t_c1 = cp.tile([P, 1], F32, tag="c1")
            nc.scalar.dma_start(t_c1[:], d_c1.ap()[:, :])
            t_b2 = cp.tile([P, 1], F32, tag="b2")
            nc.scalar.dma_start(t_b2[:], d_b2.ap()[:, :])
            t_own = cp.tile([P, NPC], BF16, tag="own")
            nc.scalar.dma_start(t_own[:], d_own.ap()[:, :])

            # big slab stream on the sync ring
            t_exp = cp.tile([P, S1 * P], E3, tag="exp")
            for s in range(NSEC1):
                a = int(csum[bounds[s]]) * P
                b = int(csum[bounds[s + 1]]) * P
                if b > a:
                    nc.sync.dma_start(t_exp[:, a:b], d_exp.ap()[:, a:b])

            t_y2all = cp.tile([P, CPC * P], BF16, tag="y2all")
            t_hall = cp.tile([P, CPC * P], BF16, tag="hall")

            # PE ramp warmup (no DMA dependency)
            t_warm = cp.tile([P, P], BF16, tag="warm")
            nc.vector.memset(t_warm[:], 1.0)
            ps_w = pW.tile([P, P], F32)
            for w in range(40):
                nc.tensor.matmul(out=ps_w[:], lhsT=t_warm[:],
                                 rhs=t_warm[:], start=(w == 0),
                                 stop=(w == 39))

            for pos in range(CPC):
                ci = int(order[pos])
                k = int(K1[ci])
                c0 = int(csum[pos])
                sl = lambda j0, j1: t_exp[:, (c0 + j0) * P:(c0 + j1) * P]
                ps = pA.tile([P, C_HID], F32)
                if ci in dve1 and k >= 4:
                    # DVE pre-reduction: fold k slabs -> one bf16 column
                    t_ds = dp.tile([P, P], BF16)
                    scr = sp.tile([P, scrw * P], F32)
                    for op in _fold_schedule(k):
                        if op[0] == "L0":
                            h = op[1]
                            nc.vector.tensor_tensor(
                                out=scr[:, :h * P], in0=sl(0, h),
                                in1=sl(h, 2 * h), op=OP.add)
                        elif op[0] == "odd":
                            m = op[1]
                            nc.vector.tensor_tensor(
                                out=scr[:, :P], in0=scr[:, :P],
                                in1=scr[:, (m - 1) * P:m * P], op=OP.add)
                        elif op[0] == "fold":
                            h = op[1]
                            nc.vector.tensor_tensor(
                                out=scr[:, :h * P], in0=scr[:, :h * P],
                                in1=scr[:, h * P:2 * h * P], op=OP.add)
                        else:  # final
                            nc.vector.tensor_tensor(
                                out=t_ds[:], in0=scr[:, :P],
                                in1=scr[:, P:2 * P], op=OP.add)
                    nc.tensor.matmul(out=ps[:], lhsT=t_w1[:, :C_HID],
                                     rhs=t_ds[:], start=True, stop=False)
                elif ci in dve1 and k == 2:
                    t_ds = dp.tile([P, P], BF16)
                    nc.vector.tensor_tensor(out=t_ds[:], in0=sl(0, 1),
                                            in1=sl(1, 2), op=OP.add)
                    nc.tensor.matmul(out=ps[:], lhsT=t_w1[:, :C_HID],
                                     rhs=t_ds[:], start=True, stop=False)
                else:
                    for kk in range(k):
                        nc.tensor.matmul(out=ps[:], lhsT=t_w1[:, :C_HID],
                                         rhs=sl(kk, kk + 1),
                                         start=(kk == 0), stop=False)
                nc.tensor.matmul(out=ps[:], lhsT=t_w1[:, C_HID:],
                                 rhs=t_own[:, pos * P:(pos + 1) * P],
                                 start=False, stop=True)
                # h = Relu(psum * 2^-s + c1)
                h_sl = t_hall[:, pos * P:(pos + 1) * P]
                nc.scalar.activation(out=h_sl, in_=ps[:], func=AF.Relu,
                                     bias=t_c1[:, 0:1],
                                     scale=float(1.0 / s1[ci]))
                # fused projection: psum2 = [W2l|W2r]^T h ; y2 = psum2 + [0;b2]
                ps2 = pP.tile([P, P], F32)
                nc.tensor.matmul(out=ps2[:], lhsT=t_w2[:], rhs=h_sl,
                                 start=True, stop=True)
                y_sl = t_y2all[:, pos * P:(pos + 1) * P]
                nc.scalar.activation(out=y_sl, in_=ps2[:], func=AF.Identity,
                                     bias=t_b2[:, 0:1], scale=1.0)
                if pos in (15, 31, 43, CPC - 1):
                    prev = {15: 0, 31: 16, 43: 32, CPC - 1: 44}[pos]
                    nc.scalar.dma_start(
                        d_y2.ap()[:, prev * P:(pos + 1) * P],
                        t_y2all[:, prev * P:(pos + 1) * P])

    nc.compile()
    return nc


def build_layer2(pp):
    K2 = pp["K2"]; order = pp["order"]; csum = pp["csum2"]
    dve2 = pp["dve2"]; s2 = pp["s2"]
    S2 = pp["S2"]
    scrw = max((int(K2[ci]) // 2 for ci in dve2), default=1)
    bounds = _sections(csum, NSEC2, CPC)

    nc = _mk_nc()
    d_exp = nc.dram_tensor("expT", (P, S2 * P), E3, kind="ExternalInput")
    d_y2r = nc.dram_tensor("y2rT", (HP, NPC), BF16, kind="ExternalInput")
    d_eye = nc.dram_tensor("eye", (P, HP), BF16, kind="ExternalInput")
    d_out = nc.dram_tensor("out", (HP, CPC * P), F32, kind="ExternalOutput")

    with tile.TileContext(nc) as tc:
        with (
            tc.tile_pool(name="const", bufs=1) as cp,
            tc.tile_pool(name="dsum", bufs=4) as dp,
            tc.tile_pool(name="scr", bufs=2) as sp,
            tc.tile_pool(name="psA", bufs=6, space="PSUM") as pA,
            tc.tile_pool(name="psW", bufs=1, space="PSUM") as pW,
        ):
            t_eye = cp.tile([P, HP], BF16, tag="eye")   # [I64;I64]
            nc.scalar.dma_start(t_eye[:], d_eye.ap()[:, :])
            t_y2r = cp.tile([HP, NPC], BF16, tag="y2r")
            nc.scalar.dma_start(t_y2r[:], d_y2r.ap()[:, :])

            t_exp = cp.tile([P, S2 * P], E3, tag="exp")
            for s in range(NSEC2):
                a = int(csum[bounds[s]]) * P
                b = int(csum[bounds[s + 1]]) * P
                if b > a:
                    nc.sync.dma_start(t_exp[:, a:b], d_exp.ap()[:, a:b])

            t_out = cp.tile([HP, CPC * P], F32, tag="outall")

            t_warm = cp.tile([P, P], BF16, tag="warm")
            nc.vector.memset(t_warm[:], 1.0)
            ps_w = pW.tile([P, P], F32)
            for w in range(40):
                nc.tensor.matmul(out=ps_w[:], lhsT=t_warm[:],
                                 rhs=t_warm[:], start=(w == 0),
                                 stop=(w == 39))

            for pos in range(CPC):
                ci = int(order[pos])
                k = int(K2[ci])
                c0 = int(csum[pos])
                sl = lambda j0, j1: t_exp[:, (c0 + j0) * P:(c0 + j1) * P]
                ps = pA.tile([HP, P], F32)
                if ci in dve2 and k >= 4:
                    t_ds = dp.tile([P, P], BF16)
                    scr = sp.tile([P, scrw * P], F32)
                    for op in _fold_schedule(k):
                        if op[0] == "L0":
                            h = op[1]
                            nc.vector.tensor_tensor(
                                out=scr[:, :h * P], in0=sl(0, h),
                                in1=sl(h, 2 * h), op=OP.add)
                        elif op[0] == "odd":
                            m = op[1]
                            nc.vector.tensor_tensor(
                                out=scr[:, :P], in0=scr[:, :P],
                                in1=scr[:, (m - 1) * P:m * P], op=OP.add)
                        elif op[0] == "fold":
                            h = op[1]
                            nc.vector.tensor_tensor(
                                out=scr[:, :h * P], in0=scr[:, :h * P],
                                in1=scr[:, h * P:2 * h * P], op=OP.add)
                        else:
                            nc.vector.tensor_tensor(
                                out=t_ds[:], in0=scr[:, :P],
                                in1=scr[:, P:2 * P], op=OP.add)
                    nc.tensor.matmul(out=ps[:], lhsT=t_eye[:], rhs=t_ds[:],
                                     start=True, stop=False)
                elif ci in dve2 and k == 2:
                    t_ds = dp.tile([P, P], BF16)
                    nc.vector.tensor_tensor(out=t_ds[:], in0=sl(0, 1),
                                            in1=sl(1, 2), op=OP.add)
                    nc.tensor.matmul(out=ps[:], lhsT=t_eye[:], rhs=t_ds[:],
                                     start=True, stop=False)
                else:
                    for kk in range(k):
                        nc.tensor.matmul(out=ps[:], lhsT=t_eye[:],
                                         rhs=sl(kk, kk + 1),
                                         start=(kk == 0), stop=False)
                # own y2r via I64 (top half of eye)
                nc.tensor.matmul(out=ps[:], lhsT=t_eye[:HP, :],
                                 rhs=t_y2r[:, pos * P:(pos + 1) * P],
                                 start=False, stop=True)
                o_sl = t_out[:, pos * P:(pos + 1) * P]
                nc.scalar.activation(out=o_sl, in_=ps[:], func=AF.Identity,
                                     scale=float(1.0 / s2[ci]))
                if pos in (15, 31, 43, CPC - 1):
                    prev = {15: 0, 31: 16, 43: 32, CPC - 1: 44}[pos]
                    nc.scalar.dma_start(
                        d_out.ap()[:, prev * P:(pos + 1) * P],
                        t_out[:, prev * P:(pos + 1) * P])

    nc.compile()
    return nc


def _expand8(tabT_ext, slot_idx, scale, smax=15.5):
    """tabT_ext f32 [C, NP_PAD+1] (last col 0), slot_idx int (-1 pad),
    scale f32 per slot; clip + single e3m4 rounding."""
    idx = np.where(slot_idx < 0, NP_PAD, slot_idx)
    e = tabT_ext[:, idx] * scale[None, :]
    np.clip(e, -smax, smax, out=e)
    return np.ascontiguousarray(e.astype(E3_NP))


class _EmuResults:
    """Duck-type for BassKernelResults in numpy-emulation mode."""

    def __init__(self, results):
        self.results = results
        self.exec_time_ns = None
        self.mean_exec_time_ns = None
        self.max_exec_time_core_id = None


def _emu_layer1(pp, m):
    order = pp["order"]; K1 = pp["K1"]; csum = pp["csum1"]; s1 = pp["s1"]
    expT = m["expT"].astype(np.float32)
    own = m["ownT"].astype(np.float32)
    w1 = m["w1"].astype(np.float32)
    w2 = m["w2"].astype(np.float32)
    c1 = m["c1"]; b2 = m["b2"]
    y2 = np.zeros((P, CPC * P), BF16_NP)
    for pos in range(CPC):
        ci = int(order[pos]); k = int(K1[ci]); c0 = int(csum[pos])
        slabs = expT[:, c0 * P:(c0 + k) * P].reshape(P, k, P)
        if ci in pp["dve1"]:
            ssum = slabs.sum(axis=1).astype(BF16_NP).astype(np.float32)
            ps = w1[:, :C_HID].T @ ssum
        else:
            ps = w1[:, :C_HID].T @ slabs.sum(axis=1)
        ps = ps + w1[:, C_HID:].T @ own[:, pos * P:(pos + 1) * P]
        h = np.maximum(ps * (1.0 / s1[ci]) + c1, 0).astype(BF16_NP)
        ps2 = w2.T @ h.astype(np.float32) + b2
        y2[:, pos * P:(pos + 1) * P] = ps2.astype(BF16_NP)
    return {"y2": y2}


def _emu_layer2(pp, m):
    order = pp["order"]; K2 = pp["K2"]; csum = pp["csum2"]; s2 = pp["s2"]
    expT = m["expT"].astype(np.float32)
    y2r = m["y2rT"].astype(np.float32)
    out = np.zeros((HP, CPC * P), np.float32)
    for pos in range(CPC):
        ci = int(order[pos]); k = int(K2[ci]); c0 = int(csum[pos])
        pairs = expT[:, c0 * P:(c0 + k) * P].reshape(P, k, P)
        if ci in pp["dve2"]:
            psum_pair = pairs.sum(axis=1).astype(BF16_NP).astype(np.float32)
        else:
            psum_pair = pairs.sum(axis=1)
        ps = psum_pair[:HP] + psum_pair[HP:]
        ps = ps + y2r[:, pos * P:(pos + 1) * P]
        out[:, pos * P:(pos + 1) * P] = ps * (1.0 / s2[ci])
    return {"out": out}


_EMULATE = bool(__import__("os").environ.get("KERNEL_EMULATE"))


def kernel(x, edge_index, W1_l, W1_r, b1, bn_gamma, bn_beta, bn_mean, bn_var,
           W2_l, W2_r, b2, _results=None):
    xmax = np.zeros(NP_PAD)
    xmax[:N_NODES] = np.abs(np.asarray(x, np.float32)).max(axis=1)
    pp = _preprocess(edge_index, xmax)
    nc1 = None if _EMULATE else build_layer1(pp)

    # BN folding (float64): h = gamma*(z-mean)/sqrt(var+eps)+beta
    sBN = (np.asarray(bn_gamma, np.float64)
           / np.sqrt(np.asarray(bn_var, np.float64) + BN_EPS))
    w1l_f = (np.asarray(W1_l, np.float64) * sBN[None, :]).astype(BF16_NP)
    w1r_f = (np.asarray(W1_r, np.float64) * sBN[None, :]).astype(BF16_NP)
    c1 = ((np.asarray(b1, np.float64) - np.asarray(bn_mean, np.float64)) * sBN
          + np.asarray(bn_beta, np.float64)).astype(np.float32).reshape(P, 1)
    w1 = np.ascontiguousarray(np.concatenate([w1l_f, w1r_f], axis=1))
    w2 = np.ascontiguousarray(np.concatenate(
        [np.asarray(W2_l, np.float32).astype(BF16_NP),
         np.asarray(W2_r, np.float32).astype(BF16_NP)], axis=1))
    b2col = np.concatenate([np.zeros(HP, np.float32),
                            np.asarray(b2, np.float32)]).reshape(P, 1)

    x_pad = np.zeros((NP_PAD + 1, C_IN), np.float32)
    x_pad[:N_NODES] = np.asarray(x, np.float32)
    xT_bf = np.ascontiguousarray(x_pad.astype(BF16_NP).T)   # [128, NP+1]
    xT_f = xT_bf.astype(np.float32)

    order = pp["order"]; s1 = pp["s1"]
    # own features, pre-scaled by the per-chunk pow2 (exact in bf16)
    own_scale = np.repeat(s1[order], P).astype(np.float32)

    maps1 = []
    for c in range(N_CORES):
        ownT = (xT_bf[:, pp["node_of"][c]].astype(np.float32)
                * own_scale[None, :]).astype(BF16_NP)
        maps1.append(dict(
            expT=_expand8(xT_f, pp["slot1_src"][c], pp["slot1_sc"][c]),
            ownT=np.ascontiguousarray(ownT),
            w1=w1, w2=w2, c1=c1, b2=b2col,
        ))
    if _EMULATE:
        r1 = _EmuResults([_emu_layer1(pp, m) for m in maps1])
    else:
        r1 = run_bass_kernel_spmd(nc1, maps1, list(range(N_CORES)))

    # assemble y2 (original node ids)
    y2lT = np.zeros((HP, NP_PAD + 1), BF16_NP)
    y2rT = np.zeros((HP, NP_PAD + 1), BF16_NP)
    for c in range(N_CORES):
        part = np.asarray(r1.results[c]["y2"])          # [128, CPC*128]
        y2lT[:, pp["node_of"][c]] = part[:HP]
        y2rT[:, pp["node_of"][c]] = part[HP:]
    y2lT[:, NP_PAD] = 0

    # per-chunk pow2 scales for layer-2 fp8 slabs: target |val| ~ 1.2,
    # capped so no staged value exceeds e3m4 range
    y2l_f = y2lT[:, :N_NODES].astype(np.float32)
    std = float(y2l_f.std()) + 1e-12
    y2max = np.zeros(NP_PAD)
    y2max[:N_NODES] = np.abs(y2l_f).max(axis=0)
    mx2 = np.zeros(CPC)
    np.maximum.at(mx2, pp["edge_ci"],
                  y2max[pp["edge_src"]] * pp["edge_ivd"])
    s2 = 2.0 ** np.round(np.log2(1.2 * np.maximum(pp["degmed"], 1.0) / std))
    for ci in range(CPC):
        while mx2[ci] * s2[ci] > 14.0:
            s2[ci] /= 2.0
    pp["s2"] = s2
    nc2 = None if _EMULATE else build_layer2(pp)

    y2l_ext = y2lT.astype(np.float32)
    maps2 = []
    eye = np.ascontiguousarray(
        np.concatenate([np.eye(HP), np.eye(HP)], axis=0).astype(BF16_NP))
    own2_scale = np.repeat(s2[order], P).astype(np.float32)
    slot2_s2 = np.repeat(s2[order], pp["K2"][order] * P).astype(np.float32)
    for c in range(N_CORES):
        st, sb = pp["slot2_sc"][c]
        at, ab = pp["slot2_src"][c]
        top = _expand8(y2l_ext, at, st * slot2_s2)
        bot = _expand8(y2l_ext, ab, sb * slot2_s2)
        expT2 = np.ascontiguousarray(np.concatenate([top, bot], axis=0))
        y2r_own = (y2rT[:, pp["node_of"][c]].astype(np.float32)
                   * own2_scale[None, :]).astype(BF16_NP)
        maps2.append(dict(
            expT=expT2, y2rT=np.ascontiguousarray(y2r_own), eye=eye,
        ))
    if _EMULATE:
        r2 = _EmuResults([_emu_layer2(pp, m) for m in maps2])
    else:
        r2 = run_bass_kernel_spmd(nc2, maps2, list(range(N_CORES)))

    out = np.zeros((NP_PAD, C_OUT), np.float32)
    for c in range(N_CORES):
        part = np.asarray(r2.results[c]["out"])         # [64, CPC*128]
        out[pp["node_of"][c]] = part.T
    if _results is not None:
        _results.extend([r1, r2])
    return np.ascontiguousarray(out[:N_NODES])


# revision 25
# speedup vs baseline: 1.2903x; 1.2133x over previous
"""2-layer GraphSAGE (mean aggr + BN(eval) + ReLU) on Trainium2, 8-core SPMD.

Strategy (dst-node sharding, host-mediated all-to-all, fp8 slabs, grouped
full-bank psum pipeline):
  - Host: relabel nodes by in-degree (desc), deal 128-node chunks round-robin
    to 8 cores (chunk ci has ~equal degrees on every core -> shared pad depth
    K[ci], SPMD). Consecutive chunks with equal (K, scale) form GROUPS of up
    to 4; each group owns a full PSUM bank [128, gsz*128] so the
    PE->ACT->PE pipeline never shares banks (per-chunk psum tiles caused
    bank-conflict serialization at ~1.1us/chunk).
  - Slabs are fp8-e3m4, pre-scaled by invdeg * 2^s(ci) (per-chunk pow2,
    capped so nothing clips; inverse applied by ACT at psum readout). Slot
    layout is k-major within a group, so ONE matmul per k covers the whole
    group (moving [128ch, gsz*128]).
  - Layer 1: W1 rides STATIONARY in the PE; slabs stream as moving operand.
    High-K groups are pre-reduced on the Vector engine (fold-in-half tree,
    f32 scratch, bf16 final). Per group:
       psum[chout, g*dst] = sum_k W1l^T slab_k (+ W1l^T dve_sum)
                          + W1r^T own          (own = x*2^s, bf16)
       h  = ACT(Relu, scale=2^-s, bias=c1)     (BN folded into W1/c1)
       psum2 = [W2l|W2r]^T h                   (one fused projection matmul)
       y2 = psum2 + [0;b2]                     (GPSIMD tensor_scalar_add)
    Only y2 ([y2l;y2r], bf16) returns to the host - h never does.
  - Host: regather of y2l into layer-2 slabs: fp8 stacked PAIRS ([2x64ch])
    pre-scaled by invdeg * 2^s2(ci); y2r (own dst, includes b2) stays bf16,
    pre-scaled by 2^s2(ci).
  - Layer 2: aggregation is a pure sum: stacked pairs contract with a
    constant [I64;I64] stationary; DVE pre-folds high-K groups; y2r joins
    via an I64 matmul; ACT scales by 2^-s2 to f32 out. No weights on device.
"""

import os

import numpy as np

import concourse.bacc as bacc
import concourse.mybir as mybir
import concourse.tile as tile
from concourse.bass_utils import run_bass_kernel_spmd

F32 = mybir.dt.float32
BF16 = mybir.dt.bfloat16
E3 = mybir.dt.float8e3
OP = mybir.AluOpType
AF = mybir.ActivationFunctionType
BF16_NP = mybir.dt.np(mybir.dt.bfloat16)
E3_NP = mybir.dt.np(mybir.dt.float8e3)

N_CORES = 8
P = 128
HP = 64

N_NODES = 50000
NP_PAD = 50176            # 392 chunks of 128
C_IN, C_HID, C_OUT = 128, 128, 64
CPC = NP_PAD // P // N_CORES   # 49 chunks per core
NPC = CPC * P                  # 6272 nodes per core
BN_EPS = 1e-5

# tuning knobs
DVE_SLOT_BUDGET_L1 = 200       # ~ slots pre-reduced on DVE in layer 1
DVE_PAIR_BUDGET_L2 = 80        # ~ pair-columns pre-reduced on DVE in layer 2
GROUP_MAX = 4
NSEC1 = 16
NSEC2 = 10
_EMULATE = bool(os.environ.get("KERNEL_EMULATE"))


def _fold_schedule(m):
    """Fold-in-half schedule for m group-columns -> 2 (then a final add).

    ('L0', h): scr[0:h] = in[0:h] + in[h:2h]     (m even, h=m//2)
    ('odd', c): scr[0] += scr[c-1]
    ('fold', h): scr[0:h] += scr[h:2h]
    ('final',): out = scr[0] + scr[1]            (bf16)
    """
    assert m % 2 == 0 and m >= 4
    ops = [("L0", m // 2)]
    m //= 2
    while m > 2:
        if m % 2 == 1:
            ops.append(("odd", m))
            m -= 1
        if m == 2:
            break
        ops.append(("fold", m // 2))
        m //= 2
    ops.append(("final",))
    return ops


def _make_groups(Kv, sv, budget, min_fold=2):
    """Group consecutive chunks (K-desc chunk ids) with equal (K, scale),
    size<=GROUP_MAX; mark top-K groups as DVE until slot budget is used;
    interleave DVE among PE groups for engine overlap.

    Returns list of dicts: chunks, K (padded even for DVE), s, dve, plus
    storage fields pos0/base filled later.
    """
    groups = []
    i = 0
    while i < CPC:
        j = i
        while (j < CPC and j - i < GROUP_MAX and Kv[j] == Kv[i]
               and sv[j] == sv[i]):
            j += 1
        groups.append(dict(chunks=list(range(i, j)), K=int(Kv[i]),
                           s=float(sv[i]), dve=False))
        i = j
    tot = 0
    for g in groups:                       # groups are K-desc already
        cost = g["K"] * len(g["chunks"])
        if tot + cost <= budget and g["K"] >= min_fold:
            g["dve"] = True
            tot += cost
            if g["K"] % 2 and g["K"] > 1:
                g["K"] += 1                # even K for clean folds
    dlist = [g for g in groups if g["dve"]]
    plist = [g for g in groups if not g["dve"]]
    out = []
    di = pi = 0
    ratio = max(len(plist) / max(len(dlist), 1), 1.0)
    while di < len(dlist) or pi < len(plist):
        if di < len(dlist) and (pi >= len(plist) or pi >= ratio * di):
            out.append(dlist[di]); di += 1
        else:
            out.append(plist[pi]); pi += 1
    pos = 0
    base = 0
    for g in out:
        g["pos0"] = pos
        g["base"] = base
        pos += len(g["chunks"])
        base += g["K"] * len(g["chunks"])
    return out


def _group_maps(groups):
    """Per-chunk lookup arrays: storage pos, group id."""
    pos_of = np.empty(CPC, np.int64)
    gid_of = np.empty(CPC, np.int64)
    gsz_of = np.empty(CPC, np.int64)
    j_of = np.empty(CPC, np.int64)
    for gi, g in enumerate(groups):
        for jj, ci in enumerate(g["chunks"]):
            pos_of[ci] = g["pos0"] + jj
            gid_of[ci] = gi
            gsz_of[ci] = len(g["chunks"])
            j_of[ci] = jj
    return pos_of, gid_of, gsz_of, j_of


def _preprocess(edge_index, xmax):
    """Degree-sort relabeling, layer-1 grouping/slot maps, edge metadata."""
    src = np.asarray(edge_index[0]).astype(np.int64)
    dst = np.asarray(edge_index[1]).astype(np.int64)
    ne = src.shape[0]
    deg = np.bincount(dst, minlength=NP_PAD).astype(np.int64)

    nodeorder = np.argsort(-deg, kind="stable")        # rank -> node
    rank = np.empty(NP_PAD, np.int64)
    rank[nodeorder] = np.arange(NP_PAD)

    gdeg3 = deg[nodeorder].reshape(CPC, N_CORES, P)
    K = np.maximum(gdeg3.max(axis=(1, 2)), 1)
    degmed = np.maximum(np.median(gdeg3.reshape(CPC, -1), axis=1), 1.0)
    s1 = 2.0 ** np.round(np.log2(2.0 * degmed))
    # cap so no slab value exceeds e3m4 range
    ci_of_all = rank[dst] // P // N_CORES
    ivd_e_all = 1.0 / np.maximum(deg[dst], 1.0)
    mx1 = np.zeros(CPC)
    np.maximum.at(mx1, ci_of_all, np.asarray(xmax)[src] * ivd_e_all)
    for ci in range(CPC):
        while mx1[ci] * s1[ci] > 14.0:
            s1[ci] /= 2.0

    groups1 = _make_groups(K, s1, DVE_SLOT_BUDGET_L1)
    pos_of, gid_of, gsz_of, j_of = _group_maps(groups1)
    S1 = sum(g["K"] * len(g["chunks"]) for g in groups1)

    # edge -> (core, chunk, k, lane)
    key = rank[dst]
    eorder = np.argsort(key, kind="stable")
    r_s = key[eorder]
    src_s = src[eorder]
    starts = np.searchsorted(r_s, r_s, side="left")
    k_in = np.arange(ne) - starts
    gg = r_s // P
    core_e = gg % N_CORES
    ci_e = gg // N_CORES
    lane_e = r_s % P
    ivd_e = ivd_e_all[eorder]

    # layer-1 slot columns (k-major within group)
    J1 = (np.array([g["base"] for g in groups1])[gid_of[ci_e]]
          + k_in * gsz_of[ci_e] + j_of[ci_e]) * P + lane_e

    slot1_src, slot1_sc = [], []
    node_of = []
    # storage-ordered chunk ids
    chunk_at_pos = np.empty(CPC, np.int64)
    chunk_at_pos[pos_of] = np.arange(CPC)
    for c in range(N_CORES):
        m = core_e == c
        a = np.full(S1 * P, -1, np.int64)
        a[J1[m]] = src_s[m]
        slot1_src.append(a)
        sc = np.zeros(S1 * P, np.float32)
        sc[J1[m]] = (ivd_e[m] * s1[ci_e[m]]).astype(np.float32)
        slot1_sc.append(sc)
        idx = (chunk_at_pos[:, None] * N_CORES + c) * P + np.arange(P)[None, :]
        node_of.append(nodeorder[idx.reshape(-1)].astype(np.int64))

    return dict(K=K, s1=s1, degmed=degmed, groups1=groups1, S1=S1,
                chunk_at_pos=chunk_at_pos,
                slot1_src=slot1_src, slot1_sc=slot1_sc, node_of=node_of,
                edge=dict(core=core_e, ci=ci_e, k=k_in, lane=lane_e,
                          src=src_s, ivd=ivd_e))


def _l2_layout(pp, s2):
    """Layer-2 grouping (by (ceil(K/2), s2)) + stacked-pair slot maps."""
    K2p = (pp["K"] + 1) // 2
    groups2 = _make_groups(K2p, s2, DVE_PAIR_BUDGET_L2)
    pos_of, gid_of, gsz_of, j_of = _group_maps(groups2)
    S2 = sum(g["K"] * len(g["chunks"]) for g in groups2)
    ed = pp["edge"]
    kp = ed["k"] // 2
    half = ed["k"] % 2
    J2 = (np.array([g["base"] for g in groups2])[gid_of[ed["ci"]]]
          + kp * gsz_of[ed["ci"]] + j_of[ed["ci"]]) * P + ed["lane"]
    chunk_at_pos2 = np.empty(CPC, np.int64)
    chunk_at_pos2[pos_of] = np.arange(CPC)
    node_of2 = []
    slot2_src, slot2_sc = [], []
    for c in range(N_CORES):
        m = ed["core"] == c
        at = np.full(S2 * P, -1, np.int64)
        ab = np.full(S2 * P, -1, np.int64)
        mt = m & (half == 0)
        mb = m & (half == 1)
        at[J2[mt]] = ed["src"][mt]
        ab[J2[mb]] = ed["src"][mb]
        slot2_src.append((at, ab))
        st = np.zeros(S2 * P, np.float32)
        sb = np.zeros(S2 * P, np.float32)
        st[J2[mt]] = (ed["ivd"][mt] * s2[ed["ci"][mt]]).astype(np.float32)
        sb[J2[mb]] = (ed["ivd"][mb] * s2[ed["ci"][mb]]).astype(np.float32)
        slot2_sc.append((st, sb))
        idx = (chunk_at_pos2[:, None] * N_CORES + c) * P \
            + np.arange(P)[None, :]
        # node_of2 via the same nodeorder mapping as layer 1
    # reuse layer-1 nodeorder through chunk_at_pos2
    return dict(groups2=groups2, S2=S2, chunk_at_pos2=chunk_at_pos2,
                slot2_src=slot2_src, slot2_sc=slot2_sc)


def _mk_nc():
    return bacc.Bacc(
        "TRN2",
        target_bir_lowering=False,
        debug=False,
        enable_asserts=False,
        num_devices=N_CORES,
    )


def _sections(groups, nsec):
    """Progressive sections over the slab stream, cut at group boundaries.
    Returns list of (col_a, col_b) slot-column ranges."""
    S = sum(g["K"] * len(g["chunks"]) for g in groups)
    edges = np.cumsum([0] + [g["K"] * len(g["chunks"]) for g in groups])
    fracs = np.cumsum([0] + [1.5, 1.5, 2, 3, 5] + [7] * (nsec - 6) + [4])
    fracs = fracs / fracs[-1]
    cuts = [0]
    for s in range(1, nsec):
        b = int(np.searchsorted(edges, S * fracs[s]))
        cuts.append(min(max(b, cuts[-1]), len(groups)))
    cuts.append(len(groups))
    return [(int(edges[a]), int(edges[b])) for a, b in zip(cuts, cuts[1:])]


def _emit_fold(nc, sl, scr, t_ds, kg, W):
    """Emit DVE fold-in-half tree: kg group-columns of width W -> t_ds."""
    if kg == 2:
        nc.vector.tensor_tensor(out=t_ds[:, :W], in0=sl(0, 1), in1=sl(1, 2),
                                op=OP.add)
        return
    for op in _fold_schedule(kg):
        if op[0] == "L0":
            h = op[1]
            nc.vector.tensor_tensor(out=scr[:, :h * W], in0=sl(0, h),
                                    in1=sl(h, 2 * h), op=OP.add)
        elif op[0] == "odd":
            c = op[1]
            nc.vector.tensor_tensor(out=scr[:, :W], in0=scr[:, :W],
                                    in1=scr[:, (c - 1) * W:c * W], op=OP.add)
        elif op[0] == "fold":
            h = op[1]
            nc.vector.tensor_tensor(out=scr[:, :h * W], in0=scr[:, :h * W],
                                    in1=scr[:, h * W:2 * h * W], op=OP.add)
        else:
            nc.vector.tensor_tensor(out=t_ds[:, :W], in0=scr[:, :W],
                                    in1=scr[:, W:2 * W], op=OP.add)


def _flush_points(groups):
    """Output-stripe flush points: after groups nearest to 1/3, 2/3, end."""
    npos = [g["pos0"] + len(g["chunks"]) for g in groups]
    marks = []
    for frac in (0.4, 0.7, 0.9):
        tgt = int(CPC * frac)
        gi = int(np.argmin([abs(npos[i] - tgt) for i in range(len(npos))]))
        if gi not in marks:
            marks.append(gi)
    marks.append(len(groups) - 1)
    return marks


def build_layer1(pp):
    groups = pp["groups1"]
    s1 = pp["s1"]
    S1 = pp["S1"]
    secs = _sections(groups, NSEC1)
    scrw = max((g["K"] // 2 * len(g["chunks"]) for g in groups if g["dve"]),
               default=1)

    nc = _mk_nc()
    d_exp = nc.dram_tensor("expT", (P, S1 * P), E3, kind="ExternalInput")
    d_own = nc.dram_tensor("ownT", (P, NPC), BF16, kind="ExternalInput")
    d_w1 = nc.dram_tensor("w1", (C_IN, 2 * C_HID), BF16, kind="ExternalInput")
    d_w2 = nc.dram_tensor("w2", (C_HID, P), BF16, kind="ExternalInput")
    d_c1 = nc.dram_tensor("c1", (P, 1), F32, kind="ExternalInput")
    d_b2 = nc.dram_tensor("b2", (P, 1), F32, kind="ExternalInput")
    d_y2 = nc.dram_tensor("y2", (P, CPC * P), BF16, kind="ExternalOutput")

    flushes = _flush_points(groups)

    with tile.TileContext(nc) as tc:
        with (
            tc.tile_pool(name="const", bufs=1) as cp,
            tc.tile_pool(name="dsum", bufs=3) as dp,
            tc.tile_pool(name="scr", bufs=2) as sp,
            tc.tile_pool(name="psA", bufs=3, space="PSUM") as pA,
            tc.tile_pool(name="psP", bufs=2, space="PSUM") as pP,
            tc.tile_pool(name="psW", bufs=1, space="PSUM") as pW,
        ):
            t_exp = cp.tile([P, S1 * P], E3, tag="exp")
            for a, b in secs:
                if b > a:
                    nc.sync.dma_start(t_exp[:, a * P:b * P],
                                      d_exp.ap()[:, a * P:b * P])
            t_w1 = cp.tile([C_IN, 2 * C_HID], BF16, tag="w1")
            nc.scalar.dma_start(t_w1[:], d_w1.ap()[:, :])
            t_w2 = cp.tile([C_HID, P], BF16, tag="w2")
            nc.scalar.dma_start(t_w2[:], d_w2.ap()[:, :])
            t_c1 = cp.tile([P, 1], F32, tag="c1")
            nc.scalar.dma_start(t_c1[:], d_c1.ap()[:, :])
            t_b2 = cp.tile([P, 1], F32, tag="b2")
            nc.scalar.dma_start(t_b2[:], d_b2.ap()[:, :])
            t_own = cp.tile([P, NPC], BF16, tag="own")
            nc.scalar.dma_start(t_own[:], d_own.ap()[:, :])

            t_y2all = cp.tile([P, CPC * P], BF16, tag="y2all")
            t_hall = cp.tile([P, CPC * P], BF16, tag="hall")

            t_warm = cp.tile([P, P], BF16, tag="warm")
            nc.vector.memset(t_warm[:], 1.0)
            ps_w = pW.tile([P, P], F32)
            for w in range(40):
                nc.tensor.matmul(out=ps_w[:], lhsT=t_warm[:], rhs=t_warm[:],
                                 start=(w == 0), stop=(w == 39))

            pend = None          # (ps tile is consumed by ACT; proj pipelined)
            flushed = 0

            def emit_proj(g):
                gsz = len(g["chunks"])
                W = gsz * P
                p0 = g["pos0"]
                ps2 = pP.tile([P, 4 * P], F32)
                nc.tensor.matmul(out=ps2[:, :W], lhsT=t_w2[:],
                                 rhs=t_hall[:, p0 * P:p0 * P + W],
                                 start=True, stop=True)
                nc.scalar.activation(
                    out=t_y2all[:, p0 * P:p0 * P + W],
                    in_=ps2[:, :W], func=AF.Identity,
                    bias=t_b2[:, 0:1], scale=1.0)

            for gi, g in enumerate(groups):
                gsz = len(g["chunks"])
                W = gsz * P
                kg = g["K"]
                b0 = g["base"]
                p0 = g["pos0"]
                sl = lambda j0, j1: t_exp[:, (b0 + j0 * gsz) * P:
                                          (b0 + j1 * gsz) * P]
                ps = pA.tile([P, 4 * P], F32)
                if g["dve"]:
                    t_ds = dp.tile([P, 4 * P], BF16)
                    scr = sp.tile([P, scrw * P], F32)
                    _emit_fold(nc, sl, scr, t_ds, kg, W)
                    nc.tensor.matmul(out=ps[:, :W], lhsT=t_w1[:, :C_HID],
                                     rhs=t_ds[:, :W], start=True, stop=False)
                else:
                    for k in range(kg):
                        nc.tensor.matmul(out=ps[:, :W], lhsT=t_w1[:, :C_HID],
                                         rhs=sl(k, k + 1),
                                         start=(k == 0), stop=False)
                nc.tensor.matmul(out=ps[:, :W], lhsT=t_w1[:, C_HID:],
                                 rhs=t_own[:, p0 * P:p0 * P + W],
                                 start=False, stop=True)
                nc.scalar.activation(out=t_hall[:, p0 * P:p0 * P + W],
                                     in_=ps[:, :W], func=AF.Relu,
                                     bias=t_c1[:, 0:1],
                                     scale=float(1.0 / g["s"]))
                if pend is not None:
                    emit_proj(pend)
                pend = g
                if gi in flushes and p0 > flushed:
                    # groups before g are fully projected at this point
                    nc.scalar.dma_start(
                        d_y2.ap()[:, flushed * P:p0 * P],
                        t_y2all[:, flushed * P:p0 * P])
                    flushed = p0
            if pend is not None:
                emit_proj(pend)
            nc.scalar.dma_start(
                d_y2.ap()[:, flushed * P:CPC * P],
                t_y2all[:, flushed * P:CPC * P])

    nc.compile()
    return nc


def build_layer2(pp, l2):
    groups = l2["groups2"]
    S2 = l2["S2"]
    secs = _sections(groups, NSEC2)
    scrw = max((g["K"] // 2 * len(g["chunks"]) for g in groups if g["dve"]),
               default=1)

    nc = _mk_nc()
    d_exp = nc.dram_tensor("expT", (P, S2 * P), E3, kind="ExternalInput")
    d_y2r = nc.dram_tensor("y2rT", (HP, NPC), BF16, kind="ExternalInput")
    d_eye = nc.dram_tensor("eye", (P, HP), BF16, kind="ExternalInput")
    d_out = nc.dram_tensor("out", (HP, CPC * P), F32, kind="ExternalOutput")

    flushes = _flush_points(groups)

    with tile.TileContext(nc) as tc:
        with (
            tc.tile_pool(name="const", bufs=1) as cp,
            tc.tile_pool(name="dsum", bufs=3) as dp,
            tc.tile_pool(name="scr", bufs=2) as sp,
            tc.tile_pool(name="psA", bufs=4, space="PSUM") as pA,
            tc.tile_pool(name="psW", bufs=1, space="PSUM") as pW,
        ):
            t_exp = cp.tile([P, S2 * P], E3, tag="exp")
            for a, b in secs:
                if b > a:
                    nc.sync.dma_start(t_exp[:, a * P:b * P],
                                      d_exp.ap()[:, a * P:b * P])
            t_eye = cp.tile([P, HP], BF16, tag="eye")
            nc.scalar.dma_start(t_eye[:], d_eye.ap()[:, :])
            t_y2r = cp.tile([HP, NPC], BF16, tag="y2r")
            nc.scalar.dma_start(t_y2r[:], d_y2r.ap()[:, :])

            t_out = cp.tile([HP, CPC * P], F32, tag="outall")

            t_warm = cp.tile([P, P], BF16, tag="warm")
            nc.vector.memset(t_warm[:], 1.0)
            ps_w = pW.tile([P, P], F32)
            for w in range(40):
                nc.tensor.matmul(out=ps_w[:], lhsT=t_warm[:], rhs=t_warm[:],
                                 start=(w == 0), stop=(w == 39))

            flushed = 0
            for gi, g in enumerate(groups):
                gsz = len(g["chunks"])
                W = gsz * P
                kg = g["K"]
                b0 = g["base"]
                p0 = g["pos0"]
                sl = lambda j0, j1: t_exp[:, (b0 + j0 * gsz) * P:
                                          (b0 + j1 * gsz) * P]
                ps = pA.tile([HP, 4 * P], F32)
                if g["dve"]:
                    t_ds = dp.tile([P, 4 * P], BF16)
                    scr = sp.tile([P, scrw * P], F32)
                    _emit_fold(nc, sl, scr, t_ds, kg, W)
                    nc.tensor.matmul(out=ps[:, :W], lhsT=t_eye[:],
                                     rhs=t_ds[:, :W], start=True, stop=False)
                else:
                    for k in range(kg):
                        nc.tensor.matmul(out=ps[:, :W], lhsT=t_eye[:],
                                         rhs=sl(k, k + 1),
                                         start=(k == 0), stop=False)
                nc.tensor.matmul(out=ps[:, :W], lhsT=t_eye[:HP, :],
                                 rhs=t_y2r[:, p0 * P:p0 * P + W],
                                 start=False, stop=True)
                nc.scalar.activation(out=t_out[:, p0 * P:p0 * P + W],
                                     in_=ps[:, :W], func=AF.Identity,
                                     scale=float(1.0 / g["s"]))
                if gi in flushes and p0 > flushed:
                    nc.scalar.dma_start(
                        d_out.ap()[:, flushed * P:p0 * P],
                        t_out[:, flushed * P:p0 * P])
                    flushed = p0
            nc.scalar.dma_start(
                d_out.ap()[:, flushed * P:CPC * P],
                t_out[:, flushed * P:CPC * P])

    nc.compile()
    return nc


def _expand8(tabT_ext, slot_idx, scale, smax=15.5):
    idx = np.where(slot_idx < 0, NP_PAD, slot_idx)
    e = tabT_ext[:, idx] * scale[None, :]
    np.clip(e, -smax, smax, out=e)
    return np.ascontiguousarray(e.astype(E3_NP))


class _EmuResults:
    def __init__(self, results):
        self.results = results
        self.exec_time_ns = None
        self.mean_exec_time_ns = None
        self.max_exec_time_core_id = None


def _emu_l1(pp, m):
    expT = m["expT"].astype(np.float32)
    own = m["ownT"].astype(np.float32)
    w1 = m["w1"].astype(np.float32)
    w2 = m["w2"].astype(np.float32)
    c1 = m["c1"]; b2 = m["b2"]
    y2 = np.zeros((P, CPC * P), BF16_NP)
    hall = np.zeros((P, CPC * P), BF16_NP)
    for g in pp["groups1"]:
        gsz = len(g["chunks"]); W = gsz * P
        kg = g["K"]; b0 = g["base"]; p0 = g["pos0"]
        slabs = expT[:, b0 * P:(b0 + kg * gsz) * P].reshape(P, kg, W)
        ssum = slabs.sum(axis=1)
        if g["dve"]:
            ssum = ssum.astype(BF16_NP).astype(np.float32)
        ps = w1[:, :C_HID].T @ ssum \
            + w1[:, C_HID:].T @ own[:, p0 * P:p0 * P + W]
        h = np.maximum(ps * (1.0 / g["s"]) + c1, 0).astype(BF16_NP)
        hall[:, p0 * P:p0 * P + W] = h
        ps2 = w2.T @ h.astype(np.float32) + b2
        y2[:, p0 * P:p0 * P + W] = ps2.astype(BF16_NP)
    return {"y2": y2}


def _emu_l2(pp, l2, m):
    expT = m["expT"].astype(np.float32)
    y2r = m["y2rT"].astype(np.float32)
    out = np.zeros((HP, CPC * P), np.float32)
    for g in l2["groups2"]:
        gsz = len(g["chunks"]); W = gsz * P
        kg = g["K"]; b0 = g["base"]; p0 = g["pos0"]
        pairs = expT[:, b0 * P:(b0 + kg * gsz) * P].reshape(P, kg, W)
        psum_pair = pairs.sum(axis=1)
        if g["dve"]:
            psum_pair = psum_pair.astype(BF16_NP).astype(np.float32)
        ps = psum_pair[:HP] + psum_pair[HP:]
        ps = ps + y2r[:, p0 * P:p0 * P + W]
        out[:, p0 * P:p0 * P + W] = ps * (1.0 / g["s"])
    return {"out": out}


def kernel(x, edge_index, W1_l, W1_r, b1, bn_gamma, bn_beta, bn_mean, bn_var,
           W2_l, W2_r, b2, _results=None):
    xmax = np.zeros(NP_PAD)
    xmax[:N_NODES] = np.abs(np.asarray(x, np.float32)).max(axis=1)
    pp = _preprocess(edge_index, xmax)
    nc1 = None if _EMULATE else build_layer1(pp)

    sBN = (np.asarray(bn_gamma, np.float64)
           / np.sqrt(np.asarray(bn_var, np.float64) + BN_EPS))
    w1l_f = (np.asarray(W1_l, np.float64) * sBN[None, :]).astype(BF16_NP)
    w1r_f = (np.asarray(W1_r, np.float64) * sBN[None, :]).astype(BF16_NP)
    c1 = ((np.asarray(b1, np.float64) - np.asarray(bn_mean, np.float64)) * sBN
          + np.asarray(bn_beta, np.float64)).astype(np.float32).reshape(P, 1)
    w1 = np.ascontiguousarray(np.concatenate([w1l_f, w1r_f], axis=1))
    w2 = np.ascontiguousarray(np.concatenate(
        [np.asarray(W2_l, np.float32).astype(BF16_NP),
         np.asarray(W2_r, np.float32).astype(BF16_NP)], axis=1))
    b2col = np.concatenate([np.zeros(HP, np.float32),
                            np.asarray(b2, np.float32)]).reshape(P, 1)

    x_pad = np.zeros((NP_PAD + 1, C_IN), np.float32)
    x_pad[:N_NODES] = np.asarray(x, np.float32)
    xT_bf = np.ascontiguousarray(x_pad.astype(BF16_NP).T)
    xT_f = xT_bf.astype(np.float32)

    s1 = pp["s1"]
    own_scale = np.repeat(s1[pp["chunk_at_pos"]], P).astype(np.float32)

    maps1 = []
    for c in range(N_CORES):
        ownT = (xT_bf[:, pp["node_of"][c]].astype(np.float32)
                * own_scale[None, :]).astype(BF16_NP)
        maps1.append(dict(
            expT=_expand8(xT_f, pp["slot1_src"][c], pp["slot1_sc"][c]),
            ownT=np.ascontiguousarray(ownT),
            w1=w1, w2=w2, c1=c1, b2=b2col,
        ))
    if _EMULATE:
        r1 = _EmuResults([_emu_l1(pp, m) for m in maps1])
    else:
        r1 = run_bass_kernel_spmd(nc1, maps1, list(range(N_CORES)))

    y2lT = np.zeros((HP, NP_PAD + 1), BF16_NP)
    y2rT = np.zeros((HP, NP_PAD + 1), BF16_NP)
    for c in range(N_CORES):
        part = np.asarray(r1.results[c]["y2"])
        y2lT[:, pp["node_of"][c]] = part[:HP]
        y2rT[:, pp["node_of"][c]] = part[HP:]
    y2lT[:, NP_PAD] = 0

    # per-chunk pow2 scales for layer-2 slabs (clip-free)
    y2l_f = y2lT[:, :N_NODES].astype(np.float32)
    std = float(y2l_f.std()) + 1e-12
    y2max = np.zeros(NP_PAD)
    y2max[:N_NODES] = np.abs(y2l_f).max(axis=0)
    ed = pp["edge"]
    mx2 = np.zeros(CPC)
    np.maximum.at(mx2, ed["ci"], y2max[ed["src"]] * ed["ivd"])
    s2 = 2.0 ** np.round(np.log2(1.2 * np.maximum(pp["degmed"], 1.0) / std))
    for ci in range(CPC):
        while mx2[ci] * s2[ci] > 14.0:
            s2[ci] /= 2.0
    l2 = _l2_layout(pp, s2)
    nc2 = None if _EMULATE else build_layer2(pp, l2)

    y2l_ext = y2lT.astype(np.float32)
    eye = np.ascontiguousarray(
        np.concatenate([np.eye(HP), np.eye(HP)], axis=0).astype(BF16_NP))
    node_of2 = []
    # node_of for layer-2 storage order
    nodeorder_map = {}
    s2_at_pos2 = s2[l2["chunk_at_pos2"]]
    own2_scale = np.repeat(s2_at_pos2, P).astype(np.float32)
    maps2 = []
    for c in range(N_CORES):
        # rebuild node_of in layer-2 storage order
        no2 = pp["node_of"][c].reshape(CPC, P)
        # node_of is in layer-1 storage order; map chunk->layer2 pos
        by_chunk = np.empty((CPC, P), np.int64)
        by_chunk[pp["chunk_at_pos"]] = no2
        no2b = by_chunk[l2["chunk_at_pos2"]].reshape(-1)
        node_of2.append(no2b)
        st, sb = l2["slot2_sc"][c]
        at, ab = l2["slot2_src"][c]
        top = _expand8(y2l_ext, at, st)
        bot = _expand8(y2l_ext, ab, sb)
        expT2 = np.ascontiguousarray(np.concatenate([top, bot], axis=0))
        y2r_own = (y2rT[:, no2b].astype(np.float32)
                   * own2_scale[None, :]).astype(BF16_NP)
        maps2.append(dict(
            expT=expT2, y2rT=np.ascontiguousarray(y2r_own), eye=eye,
        ))
    if _EMULATE:
        r2 = _EmuResults([_emu_l2(pp, l2, m) for m in maps2])
    else:
        r2 = run_bass_kernel_spmd(nc2, maps2, list(range(N_CORES)))

    out = np.zeros((NP_PAD, C_OUT), np.float32)
    for c in range(N_CORES):
        part = np.asarray(r2.results[c]["out"])
        out[node_of2[c]] = part.T
    if _results is not None:
        _results.extend([r1, r2])
    return np.ascontiguousarray(out[:N_NODES])


# revision 28
# speedup vs baseline: 1.4597x; 1.1313x over previous
"""2-layer GraphSAGE (mean aggr + BN(eval) + ReLU) on Trainium2, 8-core SPMD.

Strategy (dst-node sharding, host-mediated all-to-all, fp8 slabs, grouped
full-bank psum pipeline):
  - Host: relabel nodes by in-degree (desc), deal 128-node chunks round-robin
    to 8 cores (chunk ci has ~equal degrees on every core -> shared pad depth
    K[ci], SPMD). Consecutive chunks with equal (K, scale) form GROUPS of up
    to 4; each group owns a full PSUM bank [128, gsz*128] so the
    PE->ACT->PE pipeline never shares banks (per-chunk psum tiles caused
    bank-conflict serialization at ~1.1us/chunk).
  - Slabs are fp8-e3m4, pre-scaled by invdeg * 2^s(ci) (per-chunk pow2,
    capped so nothing clips; inverse applied by ACT at psum readout). Slot
    layout is k-major within a group, so ONE matmul per k covers the whole
    group (moving [128ch, gsz*128]).
  - Layer 1: W1 rides STATIONARY in the PE; slabs stream as moving operand.
    High-K groups are pre-reduced on the Vector engine (fold-in-half tree,
    f32 scratch, bf16 final). Per group:
       psum[chout, g*dst] = sum_k W1l^T slab_k (+ W1l^T dve_sum)
                          + W1r^T own          (own = x*2^s, bf16)
       h  = ACT(Relu, scale=2^-s, bias=c1)     (BN folded into W1/c1)
       psum2 = [W2l|W2r]^T h                   (one fused projection matmul)
       y2 = psum2 + [0;b2]                     (GPSIMD tensor_scalar_add)
    Only y2 ([y2l;y2r], bf16) returns to the host - h never does.
  - Host: regather of y2l into layer-2 slabs: fp8 stacked PAIRS ([2x64ch])
    pre-scaled by invdeg * 2^s2(ci); y2r (own dst, includes b2) stays bf16,
    pre-scaled by 2^s2(ci).
  - Layer 2: aggregation is a pure sum: stacked pairs contract with a
    constant [I64;I64] stationary; DVE pre-folds high-K groups; y2r joins
    via an I64 matmul; ACT scales by 2^-s2 to f32 out. No weights on device.
"""

import os

import numpy as np

import concourse.bacc as bacc
import concourse.mybir as mybir
import concourse.tile as tile
from concourse.bass_utils import run_bass_kernel_spmd

F32 = mybir.dt.float32
BF16 = mybir.dt.bfloat16
E3 = mybir.dt.float8e3
OP = mybir.AluOpType
AF = mybir.ActivationFunctionType
BF16_NP = mybir.dt.np(mybir.dt.bfloat16)
E3_NP = mybir.dt.np(mybir.dt.float8e3)

N_CORES = 8
P = 128
HP = 64

N_NODES = 50000
NP_PAD = 50176            # 392 chunks of 128
C_IN, C_HID, C_OUT = 128, 128, 64
CPC = NP_PAD // P // N_CORES   # 49 chunks per core
NPC = CPC * P                  # 6272 nodes per core
BN_EPS = 1e-5

# tuning knobs
DVE_SLOT_BUDGET_L1 = 160       # ~ slots pre-reduced on DVE in layer 1
DVE_PAIR_BUDGET_L2 = 70        # ~ pair-columns pre-reduced on DVE in layer 2
GROUP_MAX = 4
NSEC1 = 16
NSEC2 = 10
_EMULATE = bool(os.environ.get("KERNEL_EMULATE"))


def _fold_schedule(m):
    """Fold-in-half schedule for m group-columns -> 2 (then a final add).

    ('L0', h): scr[0:h] = in[0:h] + in[h:2h]     (m even, h=m//2)
    ('odd', c): scr[0] += scr[c-1]
    ('fold', h): scr[0:h] += scr[h:2h]
    ('final',): out = scr[0] + scr[1]            (bf16)
    """
    assert m % 2 == 0 and m >= 4
    ops = [("L0", m // 2)]
    m //= 2
    while m > 2:
        if m % 2 == 1:
            ops.append(("odd", m))
            m -= 1
        if m == 2:
            break
        ops.append(("fold", m // 2))
        m //= 2
    ops.append(("final",))
    return ops


def _make_groups(Kv, sv, budget, min_fold=2):
    """Group consecutive chunks (K-desc chunk ids) with equal (K, scale),
    size<=GROUP_MAX; mark top-K groups as DVE until slot budget is used;
    interleave DVE among PE groups for engine overlap.

    Returns list of dicts: chunks, K (padded even for DVE), s, dve, plus
    storage fields pos0/base filled later.
    """
    groups = []
    i = 0
    while i < CPC:
        j = i
        while (j < CPC and j - i < GROUP_MAX and Kv[j] == Kv[i]
               and sv[j] == sv[i]):
            j += 1
        groups.append(dict(chunks=list(range(i, j)), K=int(Kv[i]),
                           s=float(sv[i]), dve=False))
        i = j
    tot = 0
    for g in groups:                       # groups are K-desc already
        cost = g["K"] * len(g["chunks"])
        if tot + cost <= budget and g["K"] >= min_fold:
            g["dve"] = True
            tot += cost
            if g["K"] % 2 and g["K"] > 1:
                g["K"] += 1                # even K for clean folds
    dlist = [g for g in groups if g["dve"]]
    plist = [g for g in groups if not g["dve"]]
    out = []
    di = pi = 0
    ratio = max(len(plist) / max(len(dlist), 1), 1.0)
    while di < len(dlist) or pi < len(plist):
        if di < len(dlist) and (pi >= len(plist) or pi >= ratio * di):
            out.append(dlist[di]); di += 1
        else:
            out.append(plist[pi]); pi += 1
    pos = 0
    base = 0
    for g in out:
        g["pos0"] = pos
        g["base"] = base
        pos += len(g["chunks"])
        base += g["K"] * len(g["chunks"])
    return out


def _group_maps(groups):
    """Per-chunk lookup arrays: storage pos, group id."""
    pos_of = np.empty(CPC, np.int64)
    gid_of = np.empty(CPC, np.int64)
    gsz_of = np.empty(CPC, np.int64)
    j_of = np.empty(CPC, np.int64)
    for gi, g in enumerate(groups):
        for jj, ci in enumerate(g["chunks"]):
            pos_of[ci] = g["pos0"] + jj
            gid_of[ci] = gi
            gsz_of[ci] = len(g["chunks"])
            j_of[ci] = jj
    return pos_of, gid_of, gsz_of, j_of


def _preprocess(edge_index, xmax):
    """Degree-sort relabeling, layer-1 grouping/slot maps, edge metadata."""
    src = np.asarray(edge_index[0]).astype(np.int64)
    dst = np.asarray(edge_index[1]).astype(np.int64)
    ne = src.shape[0]
    deg = np.bincount(dst, minlength=NP_PAD).astype(np.int64)

    nodeorder = np.argsort(-deg, kind="stable")        # rank -> node
    rank = np.empty(NP_PAD, np.int64)
    rank[nodeorder] = np.arange(NP_PAD)

    gdeg3 = deg[nodeorder].reshape(CPC, N_CORES, P)
    K = np.maximum(gdeg3.max(axis=(1, 2)), 1)
    degmed = np.maximum(np.median(gdeg3.reshape(CPC, -1), axis=1), 1.0)
    s1 = 2.0 ** np.round(np.log2(2.0 * degmed))
    # cap so no slab value exceeds e3m4 range
    ci_of_all = rank[dst] // P // N_CORES
    ivd_e_all = 1.0 / np.maximum(deg[dst], 1.0)
    mx1 = np.zeros(CPC)
    np.maximum.at(mx1, ci_of_all, np.asarray(xmax)[src] * ivd_e_all)
    for ci in range(CPC):
        while mx1[ci] * s1[ci] > 14.0:
            s1[ci] /= 2.0

    groups1 = _make_groups(K, s1, DVE_SLOT_BUDGET_L1)
    pos_of, gid_of, gsz_of, j_of = _group_maps(groups1)
    S1 = sum(g["K"] * len(g["chunks"]) for g in groups1)

    # edge -> (core, chunk, k, lane)
    key = rank[dst]
    eorder = np.argsort(key, kind="stable")
    r_s = key[eorder]
    src_s = src[eorder]
    starts = np.searchsorted(r_s, r_s, side="left")
    k_in = np.arange(ne) - starts
    gg = r_s // P
    core_e = gg % N_CORES
    ci_e = gg // N_CORES
    lane_e = r_s % P
    ivd_e = ivd_e_all[eorder]

    # layer-1 slot columns (k-major within group)
    J1 = (np.array([g["base"] for g in groups1])[gid_of[ci_e]]
          + k_in * gsz_of[ci_e] + j_of[ci_e]) * P + lane_e

    slot1_src, slot1_sc = [], []
    node_of = []
    # storage-ordered chunk ids
    chunk_at_pos = np.empty(CPC, np.int64)
    chunk_at_pos[pos_of] = np.arange(CPC)
    for c in range(N_CORES):
        m = core_e == c
        a = np.full(S1 * P, -1, np.int64)
        a[J1[m]] = src_s[m]
        slot1_src.append(a)
        sc = np.zeros(S1 * P, np.float32)
        sc[J1[m]] = (ivd_e[m] * s1[ci_e[m]]).astype(np.float32)
        slot1_sc.append(sc)
        idx = (chunk_at_pos[:, None] * N_CORES + c) * P + np.arange(P)[None, :]
        node_of.append(nodeorder[idx.reshape(-1)].astype(np.int64))

    return dict(K=K, s1=s1, degmed=degmed, groups1=groups1, S1=S1,
                chunk_at_pos=chunk_at_pos,
                slot1_src=slot1_src, slot1_sc=slot1_sc, node_of=node_of,
                edge=dict(core=core_e, ci=ci_e, k=k_in, lane=lane_e,
                          src=src_s, ivd=ivd_e))


def _l2_layout(pp, s2):
    """Layer-2 grouping (by (ceil(K/2), s2)) + stacked-pair slot maps."""
    K2p = (pp["K"] + 1) // 2
    groups2 = _make_groups(K2p, s2, DVE_PAIR_BUDGET_L2)
    pos_of, gid_of, gsz_of, j_of = _group_maps(groups2)
    S2 = sum(g["K"] * len(g["chunks"]) for g in groups2)
    ed = pp["edge"]
    kp = ed["k"] // 2
    half = ed["k"] % 2
    J2 = (np.array([g["base"] for g in groups2])[gid_of[ed["ci"]]]
          + kp * gsz_of[ed["ci"]] + j_of[ed["ci"]]) * P + ed["lane"]
    chunk_at_pos2 = np.empty(CPC, np.int64)
    chunk_at_pos2[pos_of] = np.arange(CPC)
    node_of2 = []
    slot2_src, slot2_sc = [], []
    for c in range(N_CORES):
        m = ed["core"] == c
        at = np.full(S2 * P, -1, np.int64)
        ab = np.full(S2 * P, -1, np.int64)
        mt = m & (half == 0)
        mb = m & (half == 1)
        at[J2[mt]] = ed["src"][mt]
        ab[J2[mb]] = ed["src"][mb]
        slot2_src.append((at, ab))
        st = np.zeros(S2 * P, np.float32)
        sb = np.zeros(S2 * P, np.float32)
        st[J2[mt]] = (ed["ivd"][mt] * s2[ed["ci"][mt]]).astype(np.float32)
        sb[J2[mb]] = (ed["ivd"][mb] * s2[ed["ci"][mb]]).astype(np.float32)
        slot2_sc.append((st, sb))
        idx = (chunk_at_pos2[:, None] * N_CORES + c) * P \
            + np.arange(P)[None, :]
        # node_of2 via the same nodeorder mapping as layer 1
    # reuse layer-1 nodeorder through chunk_at_pos2
    return dict(groups2=groups2, S2=S2, chunk_at_pos2=chunk_at_pos2,
                slot2_src=slot2_src, slot2_sc=slot2_sc)


def _mk_nc():
    return bacc.Bacc(
        "TRN2",
        target_bir_lowering=False,
        debug=False,
        enable_asserts=False,
        num_devices=N_CORES,
    )


def _sections(groups, nsec):
    """Progressive sections over the slab stream, cut at group boundaries.
    Returns list of (col_a, col_b) slot-column ranges."""
    S = sum(g["K"] * len(g["chunks"]) for g in groups)
    edges = np.cumsum([0] + [g["K"] * len(g["chunks"]) for g in groups])
    fracs = np.cumsum([0] + [1.5, 1.5, 2, 3, 5] + [7] * (nsec - 6) + [4])
    fracs = fracs / fracs[-1]
    cuts = [0]
    for s in range(1, nsec):
        b = int(np.searchsorted(edges, S * fracs[s]))
        cuts.append(min(max(b, cuts[-1]), len(groups)))
    cuts.append(len(groups))
    return [(int(edges[a]), int(edges[b])) for a, b in zip(cuts, cuts[1:])]


def _emit_fold(nc, sl, scr, t_ds, kg, W):
    """Emit DVE fold-in-half tree: kg group-columns of width W -> t_ds."""
    if kg == 2:
        nc.vector.tensor_tensor(out=t_ds[:, :W], in0=sl(0, 1), in1=sl(1, 2),
                                op=OP.add)
        return
    for op in _fold_schedule(kg):
        if op[0] == "L0":
            h = op[1]
            nc.vector.tensor_tensor(out=scr[:, :h * W], in0=sl(0, h),
                                    in1=sl(h, 2 * h), op=OP.add)
        elif op[0] == "odd":
            c = op[1]
            nc.vector.tensor_tensor(out=scr[:, :W], in0=scr[:, :W],
                                    in1=scr[:, (c - 1) * W:c * W], op=OP.add)
        elif op[0] == "fold":
            h = op[1]
            nc.vector.tensor_tensor(out=scr[:, :h * W], in0=scr[:, :h * W],
                                    in1=scr[:, h * W:2 * h * W], op=OP.add)
        else:
            nc.vector.tensor_tensor(out=t_ds[:, :W], in0=scr[:, :W],
                                    in1=scr[:, W:2 * W], op=OP.add)


def _flush_points(groups):
    """Output-stripe flush points: after groups nearest to 1/3, 2/3, end."""
    npos = [g["pos0"] + len(g["chunks"]) for g in groups]
    marks = []
    for frac in (0.4, 0.7, 0.9):
        tgt = int(CPC * frac)
        gi = int(np.argmin([abs(npos[i] - tgt) for i in range(len(npos))]))
        if gi not in marks:
            marks.append(gi)
    marks.append(len(groups) - 1)
    return marks


def build_layer1(pp):
    groups = pp["groups1"]
    s1 = pp["s1"]
    S1 = pp["S1"]
    secs = _sections(groups, NSEC1)
    scrw = max((g["K"] // 2 * len(g["chunks"]) for g in groups if g["dve"]),
               default=1)

    nc = _mk_nc()
    d_exp = nc.dram_tensor("expT", (P, S1 * P), E3, kind="ExternalInput")
    d_own = nc.dram_tensor("ownT", (P, NPC), BF16, kind="ExternalInput")
    d_w1 = nc.dram_tensor("w1", (C_IN, 2 * C_HID), BF16, kind="ExternalInput")
    d_w2 = nc.dram_tensor("w2", (C_HID, P), BF16, kind="ExternalInput")
    d_c1 = nc.dram_tensor("c1", (P, 1), F32, kind="ExternalInput")
    d_b2 = nc.dram_tensor("b2", (P, 1), F32, kind="ExternalInput")
    d_y2 = nc.dram_tensor("y2", (P, CPC * P), BF16, kind="ExternalOutput")

    flushes = _flush_points(groups)

    with tile.TileContext(nc) as tc:
        with (
            tc.tile_pool(name="const", bufs=1) as cp,
            tc.tile_pool(name="dsum", bufs=3) as dp,
            tc.tile_pool(name="scr", bufs=2) as sp,
            tc.tile_pool(name="psA", bufs=4, space="PSUM") as pA,
            tc.tile_pool(name="psP", bufs=2, space="PSUM") as pP,
            tc.tile_pool(name="psW", bufs=1, space="PSUM") as pW,
        ):
            t_exp = cp.tile([P, S1 * P], E3, tag="exp")
            for a, b in secs:
                if b > a:
                    nc.sync.dma_start(t_exp[:, a * P:b * P],
                                      d_exp.ap()[:, a * P:b * P])
            t_w1 = cp.tile([C_IN, 2 * C_HID], BF16, tag="w1")
            nc.scalar.dma_start(t_w1[:], d_w1.ap()[:, :])
            t_w2 = cp.tile([C_HID, P], BF16, tag="w2")
            nc.scalar.dma_start(t_w2[:], d_w2.ap()[:, :])
            t_c1 = cp.tile([P, 1], F32, tag="c1")
            nc.scalar.dma_start(t_c1[:], d_c1.ap()[:, :])
            t_b2 = cp.tile([P, 1], F32, tag="b2")
            nc.scalar.dma_start(t_b2[:], d_b2.ap()[:, :])
            t_own = cp.tile([P, NPC], BF16, tag="own")
            nc.scalar.dma_start(t_own[:], d_own.ap()[:, :])

            t_y2all = cp.tile([P, CPC * P], BF16, tag="y2all")
            t_hall = cp.tile([P, CPC * P], BF16, tag="hall")

            t_warm = cp.tile([P, P], BF16, tag="warm")
            nc.vector.memset(t_warm[:], 1.0)
            ps_w = pW.tile([P, P], F32)
            for w in range(64):
                nc.tensor.matmul(out=ps_w[:], lhsT=t_warm[:], rhs=t_warm[:],
                                 start=(w == 0), stop=(w == 63))

            pend = None          # (ps tile is consumed by ACT; proj pipelined)
            flushed = 0
            deferred = []        # (gi, group, t_ds) folded, PE-part delayed
            done_upto = [0]      # positions with y2 complete (monotone map)

            def emit_proj(g):
                gsz = len(g["chunks"])
                W = gsz * P
                p0 = g["pos0"]
                ps2 = pP.tile([P, 4 * P], F32)
                nc.tensor.matmul(out=ps2[:, :W], lhsT=t_w2[:],
                                 rhs=t_hall[:, p0 * P:p0 * P + W],
                                 start=True, stop=True)
                nc.scalar.activation(
                    out=t_y2all[:, p0 * P:p0 * P + W],
                    in_=ps2[:, :W], func=AF.Identity,
                    bias=t_b2[:, 0:1], scale=1.0)

            def emit_pe(g):
                nonlocal pend
                gsz = len(g["chunks"])
                W = gsz * P
                kg = g["K"]
                b0 = g["base"]
                p0 = g["pos0"]
                sl = lambda j0, j1: t_exp[:, (b0 + j0 * gsz) * P:
                                          (b0 + j1 * gsz) * P]
                ps = pA.tile([P, 4 * P], F32)
                if g["dve"]:
                    nc.tensor.matmul(out=ps[:, :W], lhsT=t_w1[:, :C_HID],
                                     rhs=g["_ds"][:, :W],
                                     start=True, stop=False)
                else:
                    for k in range(kg):
                        nc.tensor.matmul(out=ps[:, :W], lhsT=t_w1[:, :C_HID],
                                         rhs=sl(k, k + 1),
                                         start=(k == 0), stop=False)
                nc.tensor.matmul(out=ps[:, :W], lhsT=t_w1[:, C_HID:],
                                 rhs=t_own[:, p0 * P:p0 * P + W],
                                 start=False, stop=True)
                nc.scalar.activation(out=t_hall[:, p0 * P:p0 * P + W],
                                     in_=ps[:, :W], func=AF.Relu,
                                     bias=t_c1[:, 0:1],
                                     scale=float(1.0 / g["s"]))
                if pend is not None:
                    emit_proj(pend)
                pend = g

            for gi, g in enumerate(groups):
                if g["dve"]:
                    gsz = len(g["chunks"])
                    W = gsz * P
                    b0 = g["base"]
                    kg = g["K"]
                    sl = lambda j0, j1: t_exp[:, (b0 + j0 * gsz) * P:
                                              (b0 + j1 * gsz) * P]
                    t_ds = dp.tile([P, 4 * P], BF16)
                    scr = sp.tile([P, scrw * P], F32)
                    _emit_fold(nc, sl, scr, t_ds, kg, W)
                    g["_ds"] = t_ds
                    deferred.append((gi, g))
                else:
                    emit_pe(g)
                while deferred and gi - deferred[0][0] >= 2:
                    emit_pe(deferred.pop(0)[1])
                if gi in flushes and flushed < CPC:
                    # stripe everything already projected (conservative:
                    # positions of groups emitted at least 2 steps back)
                    lo = min([d[1]["pos0"] for d in deferred] +
                             ([pend["pos0"]] if pend is not None else []) +
                             [CPC])
                    if lo > flushed:
                        nc.scalar.dma_start(
                            d_y2.ap()[:, flushed * P:lo * P],
                            t_y2all[:, flushed * P:lo * P])
                        flushed = lo
            for _, g in deferred:
                emit_pe(g)
            if pend is not None:
                emit_proj(pend)
            if flushed < CPC:
                nc.scalar.dma_start(
                    d_y2.ap()[:, flushed * P:CPC * P],
                    t_y2all[:, flushed * P:CPC * P])

    nc.compile()
    return nc


def build_layer2(pp, l2):
    groups = l2["groups2"]
    S2 = l2["S2"]
    secs = _sections(groups, NSEC2)
    scrw = max((g["K"] // 2 * len(g["chunks"]) for g in groups if g["dve"]),
               default=1)

    nc = _mk_nc()
    d_exp = nc.dram_tensor("expT", (P, S2 * P), E3, kind="ExternalInput")
    d_y2r = nc.dram_tensor("y2rT", (HP, NPC), BF16, kind="ExternalInput")
    d_eye = nc.dram_tensor("eye", (P, HP), BF16, kind="ExternalInput")
    d_out = nc.dram_tensor("out", (HP, CPC * P), F32, kind="ExternalOutput")

    flushes = _flush_points(groups)

    with tile.TileContext(nc) as tc:
        with (
            tc.tile_pool(name="const", bufs=1) as cp,
            tc.tile_pool(name="dsum", bufs=3) as dp,
            tc.tile_pool(name="scr", bufs=2) as sp,
            tc.tile_pool(name="psA", bufs=4, space="PSUM") as pA,
            tc.tile_pool(name="psW", bufs=1, space="PSUM") as pW,
        ):
            t_exp = cp.tile([P, S2 * P], E3, tag="exp")
            for a, b in secs:
                if b > a:
                    nc.sync.dma_start(t_exp[:, a * P:b * P],
                                      d_exp.ap()[:, a * P:b * P])
            t_eye = cp.tile([P, HP], BF16, tag="eye")
            nc.scalar.dma_start(t_eye[:], d_eye.ap()[:, :])
            t_y2r = cp.tile([HP, NPC], BF16, tag="y2r")
            nc.scalar.dma_start(t_y2r[:], d_y2r.ap()[:, :])

            t_out = cp.tile([HP, CPC * P], F32, tag="outall")

            t_warm = cp.tile([P, P], BF16, tag="warm")
            nc.vector.memset(t_warm[:], 1.0)
            ps_w = pW.tile([P, P], F32)
            for w in range(64):
                nc.tensor.matmul(out=ps_w[:], lhsT=t_warm[:], rhs=t_warm[:],
                                 start=(w == 0), stop=(w == 63))

            flushed = 0
            deferred = []
            alt = [0]

            def emit_pe(g):
                gsz = len(g["chunks"])
                W = gsz * P
                kg = g["K"]
                b0 = g["base"]
                p0 = g["pos0"]
                sl = lambda j0, j1: t_exp[:, (b0 + j0 * gsz) * P:
                                          (b0 + j1 * gsz) * P]
                ps = pA.tile([HP, 4 * P], F32)
                if g["dve"]:
                    nc.tensor.matmul(out=ps[:, :W], lhsT=t_eye[:],
                                     rhs=g["_ds"][:, :W],
                                     start=True, stop=False)
                else:
                    for k in range(kg):
                        nc.tensor.matmul(out=ps[:, :W], lhsT=t_eye[:],
                                         rhs=sl(k, k + 1),
                                         start=(k == 0), stop=False)
                nc.tensor.matmul(out=ps[:, :W], lhsT=t_eye[:HP, :],
                                 rhs=t_y2r[:, p0 * P:p0 * P + W],
                                 start=False, stop=True)
                if alt[0] % 2 == 0 or g["dve"]:
                    nc.scalar.activation(out=t_out[:, p0 * P:p0 * P + W],
                                         in_=ps[:, :W], func=AF.Identity,
                                         scale=float(1.0 / g["s"]))
                else:
                    nc.vector.tensor_scalar_mul(
                        out=t_out[:, p0 * P:p0 * P + W],
                        in0=ps[:, :W], scalar1=float(1.0 / g["s"]))
                alt[0] += 1

            for gi, g in enumerate(groups):
                if g["dve"]:
                    gsz = len(g["chunks"])
                    W = gsz * P
                    b0 = g["base"]
                    kg = g["K"]
                    sl = lambda j0, j1: t_exp[:, (b0 + j0 * gsz) * P:
                                              (b0 + j1 * gsz) * P]
                    t_ds = dp.tile([P, 4 * P], BF16)
                    scr = sp.tile([P, scrw * P], F32)
                    _emit_fold(nc, sl, scr, t_ds, kg, W)
                    g["_ds"] = t_ds
                    deferred.append((gi, g))
                else:
                    emit_pe(g)
                while deferred and gi - deferred[0][0] >= 2:
                    emit_pe(deferred.pop(0)[1])
                if gi in flushes and flushed < CPC:
                    lo = min([d[1]["pos0"] for d in deferred]
                             + [g["pos0"] + len(g["chunks"])])
                    if lo > flushed:
                        nc.scalar.dma_start(
                            d_out.ap()[:, flushed * P:lo * P],
                            t_out[:, flushed * P:lo * P])
                        flushed = lo
            for _, g in deferred:
                emit_pe(g)
            if flushed < CPC:
                nc.scalar.dma_start(
                    d_out.ap()[:, flushed * P:CPC * P],
                    t_out[:, flushed * P:CPC * P])

    nc.compile()
    return nc


def _expand8(tabT_ext, slot_idx, scale, smax=15.5):
    idx = np.where(slot_idx < 0, NP_PAD, slot_idx)
    e = tabT_ext[:, idx] * scale[None, :]
    np.clip(e, -smax, smax, out=e)
    return np.ascontiguousarray(e.astype(E3_NP))


class _EmuResults:
    def __init__(self, results):
        self.results = results
        self.exec_time_ns = None
        self.mean_exec_time_ns = None
        self.max_exec_time_core_id = None


def _emu_l1(pp, m):
    expT = m["expT"].astype(np.float32)
    own = m["ownT"].astype(np.float32)
    w1 = m["w1"].astype(np.float32)
    w2 = m["w2"].astype(np.float32)
    c1 = m["c1"]; b2 = m["b2"]
    y2 = np.zeros((P, CPC * P), BF16_NP)
    hall = np.zeros((P, CPC * P), BF16_NP)
    for g in pp["groups1"]:
        gsz = len(g["chunks"]); W = gsz * P
        kg = g["K"]; b0 = g["base"]; p0 = g["pos0"]
        slabs = expT[:, b0 * P:(b0 + kg * gsz) * P].reshape(P, kg, W)
        ssum = slabs.sum(axis=1)
        if g["dve"]:
            ssum = ssum.astype(BF16_NP).astype(np.float32)
        ps = w1[:, :C_HID].T @ ssum \
            + w1[:, C_HID:].T @ own[:, p0 * P:p0 * P + W]
        h = np.maximum(ps * (1.0 / g["s"]) + c1, 0).astype(BF16_NP)
        hall[:, p0 * P:p0 * P + W] = h
        ps2 = w2.T @ h.astype(np.float32) + b2
        y2[:, p0 * P:p0 * P + W] = ps2.astype(BF16_NP)
    return {"y2": y2}


def _emu_l2(pp, l2, m):
    expT = m["expT"].astype(np.float32)
    y2r = m["y2rT"].astype(np.float32)
    out = np.zeros((HP, CPC * P), np.float32)
    for g in l2["groups2"]:
        gsz = len(g["chunks"]); W = gsz * P
        kg = g["K"]; b0 = g["base"]; p0 = g["pos0"]
        pairs = expT[:, b0 * P:(b0 + kg * gsz) * P].reshape(P, kg, W)
        psum_pair = pairs.sum(axis=1)
        if g["dve"]:
            psum_pair = psum_pair.astype(BF16_NP).astype(np.float32)
        ps = psum_pair[:HP] + psum_pair[HP:]
        ps = ps + y2r[:, p0 * P:p0 * P + W]
        out[:, p0 * P:p0 * P + W] = ps * (1.0 / g["s"])
    return {"out": out}


def kernel(x, edge_index, W1_l, W1_r, b1, bn_gamma, bn_beta, bn_mean, bn_var,
           W2_l, W2_r, b2, _results=None):
    xmax = np.zeros(NP_PAD)
    xmax[:N_NODES] = np.abs(np.asarray(x, np.float32)).max(axis=1)
    pp = _preprocess(edge_index, xmax)
    nc1 = None if _EMULATE else build_layer1(pp)

    sBN = (np.asarray(bn_gamma, np.float64)
           / np.sqrt(np.asarray(bn_var, np.float64) + BN_EPS))
    w1l_f = (np.asarray(W1_l, np.float64) * sBN[None, :]).astype(BF16_NP)
    w1r_f = (np.asarray(W1_r, np.float64) * sBN[None, :]).astype(BF16_NP)
    c1 = ((np.asarray(b1, np.float64) - np.asarray(bn_mean, np.float64)) * sBN
          + np.asarray(bn_beta, np.float64)).astype(np.float32).reshape(P, 1)
    w1 = np.ascontiguousarray(np.concatenate([w1l_f, w1r_f], axis=1))
    w2 = np.ascontiguousarray(np.concatenate(
        [np.asarray(W2_l, np.float32).astype(BF16_NP),
         np.asarray(W2_r, np.float32).astype(BF16_NP)], axis=1))
    b2col = np.concatenate([np.zeros(HP, np.float32),
                            np.asarray(b2, np.float32)]).reshape(P, 1)

    x_pad = np.zeros((NP_PAD + 1, C_IN), np.float32)
    x_pad[:N_NODES] = np.asarray(x, np.float32)
    xT_bf = np.ascontiguousarray(x_pad.astype(BF16_NP).T)
    xT_f = xT_bf.astype(np.float32)

    s1 = pp["s1"]
    own_scale = np.repeat(s1[pp["chunk_at_pos"]], P).astype(np.float32)

    maps1 = []
    for c in range(N_CORES):
        ownT = (xT_bf[:, pp["node_of"][c]].astype(np.float32)
                * own_scale[None, :]).astype(BF16_NP)
        maps1.append(dict(
            expT=_expand8(xT_f, pp["slot1_src"][c], pp["slot1_sc"][c]),
            ownT=np.ascontiguousarray(ownT),
            w1=w1, w2=w2, c1=c1, b2=b2col,
        ))
    if _EMULATE:
        r1 = _EmuResults([_emu_l1(pp, m) for m in maps1])
    else:
        r1 = run_bass_kernel_spmd(nc1, maps1, list(range(N_CORES)))

    y2lT = np.zeros((HP, NP_PAD + 1), BF16_NP)
    y2rT = np.zeros((HP, NP_PAD + 1), BF16_NP)
    for c in range(N_CORES):
        part = np.asarray(r1.results[c]["y2"])
        y2lT[:, pp["node_of"][c]] = part[:HP]
        y2rT[:, pp["node_of"][c]] = part[HP:]
    y2lT[:, NP_PAD] = 0

    # per-chunk pow2 scales for layer-2 slabs (clip-free)
    y2l_f = y2lT[:, :N_NODES].astype(np.float32)
    std = float(y2l_f.std()) + 1e-12
    y2max = np.zeros(NP_PAD)
    y2max[:N_NODES] = np.abs(y2l_f).max(axis=0)
    ed = pp["edge"]
    mx2 = np.zeros(CPC)
    np.maximum.at(mx2, ed["ci"], y2max[ed["src"]] * ed["ivd"])
    s2 = 2.0 ** np.round(np.log2(1.2 * np.maximum(pp["degmed"], 1.0) / std))
    for ci in range(CPC):
        while mx2[ci] * s2[ci] > 14.0:
            s2[ci] /= 2.0
    l2 = _l2_layout(pp, s2)
    nc2 = None if _EMULATE else build_layer2(pp, l2)

    y2l_ext = y2lT.astype(np.float32)
    eye = np.ascontiguousarray(
        np.concatenate([np.eye(HP), np.eye(HP)], axis=0).astype(BF16_NP))
    node_of2 = []
    # node_of for layer-2 storage order
    nodeorder_map = {}
    s2_at_pos2 = s2[l2["chunk_at_pos2"]]
    own2_scale = np.repeat(s2_at_pos2, P).astype(np.float32)
    maps2 = []
    for c in range(N_CORES):
        # rebuild node_of in layer-2 storage order
        no2 = pp["node_of"][c].reshape(CPC, P)
        # node_of is in layer-1 storage order; map chunk->layer2 pos
        by_chunk = np.empty((CPC, P), np.int64)
        by_chunk[pp["chunk_at_pos"]] = no2
        no2b = by_chunk[l2["chunk_at_pos2"]].reshape(-1)
        node_of2.append(no2b)
        st, sb = l2["slot2_sc"][c]
        at, ab = l2["slot2_src"][c]
        top = _expand8(y2l_ext, at, st)
        bot = _expand8(y2l_ext, ab, sb)
        expT2 = np.ascontiguousarray(np.concatenate([top, bot], axis=0))
        y2r_own = (y2rT[:, no2b].astype(np.float32)
                   * own2_scale[None, :]).astype(BF16_NP)
        maps2.append(dict(
            expT=expT2, y2rT=np.ascontiguousarray(y2r_own), eye=eye,
        ))
    if _EMULATE:
        r2 = _EmuResults([_emu_l2(pp, l2, m) for m in maps2])
    else:
        r2 = run_bass_kernel_spmd(nc2, maps2, list(range(N_CORES)))

    out = np.zeros((NP_PAD, C_OUT), np.float32)
    for c in range(N_CORES):
        part = np.asarray(r2.results[c]["out"])
        out[node_of2[c]] = part.T
    if _results is not None:
        _results.extend([r1, r2])
    return np.ascontiguousarray(out[:N_NODES])


# revision 29
# speedup vs baseline: 1.4901x; 1.0208x over previous
"""2-layer GraphSAGE (mean aggr + BN(eval) + ReLU) on Trainium2, 8-core SPMD.

Strategy (dst-node sharding, host-mediated all-to-all, fp8 slabs, grouped
full-bank psum pipeline):
  - Host: relabel nodes by in-degree (desc), deal 128-node chunks round-robin
    to 8 cores (chunk ci has ~equal degrees on every core -> shared pad depth
    K[ci], SPMD). Consecutive chunks with equal (K, scale) form GROUPS of up
    to 4; each group owns a full PSUM bank [128, gsz*128] so the
    PE->ACT->PE pipeline never shares banks (per-chunk psum tiles caused
    bank-conflict serialization at ~1.1us/chunk).
  - Slabs are fp8-e3m4, pre-scaled by invdeg * 2^s(ci) (per-chunk pow2,
    capped so nothing clips; inverse applied by ACT at psum readout). Slot
    layout is k-major within a group, so ONE matmul per k covers the whole
    group (moving [128ch, gsz*128]).
  - Layer 1: W1 rides STATIONARY in the PE; slabs stream as moving operand.
    High-K groups are pre-reduced on the Vector engine (fold-in-half tree,
    f32 scratch, bf16 final). Per group:
       psum[chout, g*dst] = sum_k W1l^T slab_k (+ W1l^T dve_sum)
                          + W1r^T own          (own = x*2^s, bf16)
       h  = ACT(Relu, scale=2^-s, bias=c1)     (BN folded into W1/c1)
       psum2 = [W2l|W2r]^T h                   (one fused projection matmul)
       y2 = psum2 + [0;b2]                     (GPSIMD tensor_scalar_add)
    Only y2 ([y2l;y2r], bf16) returns to the host - h never does.
  - Host: regather of y2l into layer-2 slabs: fp8 stacked PAIRS ([2x64ch])
    pre-scaled by invdeg * 2^s2(ci); y2r (own dst, includes b2) stays bf16,
    pre-scaled by 2^s2(ci).
  - Layer 2: aggregation is a pure sum: stacked pairs contract with a
    constant [I64;I64] stationary; DVE pre-folds high-K groups; y2r joins
    via an I64 matmul; ACT scales by 2^-s2 to f32 out. No weights on device.
"""

import os

import numpy as np

import concourse.bacc as bacc
import concourse.mybir as mybir
import concourse.tile as tile
from concourse.bass_utils import run_bass_kernel_spmd

F32 = mybir.dt.float32
BF16 = mybir.dt.bfloat16
E3 = mybir.dt.float8e3
OP = mybir.AluOpType
AF = mybir.ActivationFunctionType
BF16_NP = mybir.dt.np(mybir.dt.bfloat16)
E3_NP = mybir.dt.np(mybir.dt.float8e3)

N_CORES = 8
P = 128
HP = 64

N_NODES = 50000
NP_PAD = 50176            # 392 chunks of 128
C_IN, C_HID, C_OUT = 128, 128, 64
CPC = NP_PAD // P // N_CORES   # 49 chunks per core
NPC = CPC * P                  # 6272 nodes per core
BN_EPS = 1e-5

# tuning knobs
DVE_SLOT_BUDGET_L1 = 160       # ~ slots pre-reduced on DVE in layer 1
DVE_PAIR_BUDGET_L2 = 70        # ~ pair-columns pre-reduced on DVE in layer 2
GROUP_MAX = 4
NSEC1 = 8
NSEC2 = 6
_EMULATE = bool(os.environ.get("KERNEL_EMULATE"))


def _fold_schedule(m):
    """Fold-in-half schedule for m group-columns -> 2 (then a final add).

    ('L0', h): scr[0:h] = in[0:h] + in[h:2h]     (m even, h=m//2)
    ('odd', c): scr[0] += scr[c-1]
    ('fold', h): scr[0:h] += scr[h:2h]
    ('final',): out = scr[0] + scr[1]            (bf16)
    """
    assert m % 2 == 0 and m >= 4
    ops = [("L0", m // 2)]
    m //= 2
    while m > 2:
        if m % 2 == 1:
            ops.append(("odd", m))
            m -= 1
        if m == 2:
            break
        ops.append(("fold", m // 2))
        m //= 2
    ops.append(("final",))
    return ops


def _make_groups(Kv, sv, budget, min_fold=2):
    """Group consecutive chunks (K-desc chunk ids) with equal (K, scale),
    size<=GROUP_MAX; mark top-K groups as DVE until slot budget is used;
    interleave DVE among PE groups for engine overlap.

    Returns list of dicts: chunks, K (padded even for DVE), s, dve, plus
    storage fields pos0/base filled later.
    """
    groups = []
    i = 0
    while i < CPC:
        j = i
        while (j < CPC and j - i < GROUP_MAX and Kv[j] == Kv[i]
               and sv[j] == sv[i]):
            j += 1
        groups.append(dict(chunks=list(range(i, j)), K=int(Kv[i]),
                           s=float(sv[i]), dve=False))
        i = j
    tot = 0
    for g in groups:                       # groups are K-desc already
        cost = g["K"] * len(g["chunks"])
        if tot + cost <= budget and g["K"] >= min_fold:
            g["dve"] = True
            tot += cost
            if g["K"] % 2 and g["K"] > 1:
                g["K"] += 1                # even K for clean folds
    dlist = [g for g in groups if g["dve"]]
    plist = [g for g in groups if not g["dve"]]
    out = []
    di = pi = 0
    ratio = max(len(plist) / max(len(dlist), 1), 1.0)
    while di < len(dlist) or pi < len(plist):
        if di < len(dlist) and (pi >= len(plist) or pi >= ratio * di):
            out.append(dlist[di]); di += 1
        else:
            out.append(plist[pi]); pi += 1
    pos = 0
    base = 0
    for g in out:
        g["pos0"] = pos
        g["base"] = base
        pos += len(g["chunks"])
        base += g["K"] * len(g["chunks"])
    return out


def _group_maps(groups):
    """Per-chunk lookup arrays: storage pos, group id."""
    pos_of = np.empty(CPC, np.int64)
    gid_of = np.empty(CPC, np.int64)
    gsz_of = np.empty(CPC, np.int64)
    j_of = np.empty(CPC, np.int64)
    for gi, g in enumerate(groups):
        for jj, ci in enumerate(g["chunks"]):
            pos_of[ci] = g["pos0"] + jj
            gid_of[ci] = gi
            gsz_of[ci] = len(g["chunks"])
            j_of[ci] = jj
    return pos_of, gid_of, gsz_of, j_of


def _preprocess(edge_index, xmax):
    """Degree-sort relabeling, layer-1 grouping/slot maps, edge metadata."""
    src = np.asarray(edge_index[0]).astype(np.int64)
    dst = np.asarray(edge_index[1]).astype(np.int64)
    ne = src.shape[0]
    deg = np.bincount(dst, minlength=NP_PAD).astype(np.int64)

    nodeorder = np.argsort(-deg, kind="stable")        # rank -> node
    rank = np.empty(NP_PAD, np.int64)
    rank[nodeorder] = np.arange(NP_PAD)

    gdeg3 = deg[nodeorder].reshape(CPC, N_CORES, P)
    K = np.maximum(gdeg3.max(axis=(1, 2)), 1)
    degmed = np.maximum(np.median(gdeg3.reshape(CPC, -1), axis=1), 1.0)
    s1 = 2.0 ** np.round(np.log2(2.0 * degmed))
    # cap so no slab value exceeds e3m4 range
    ci_of_all = rank[dst] // P // N_CORES
    ivd_e_all = 1.0 / np.maximum(deg[dst], 1.0)
    mx1 = np.zeros(CPC)
    np.maximum.at(mx1, ci_of_all, np.asarray(xmax)[src] * ivd_e_all)
    for ci in range(CPC):
        while mx1[ci] * s1[ci] > 14.0:
            s1[ci] /= 2.0

    groups1 = _make_groups(K, s1, DVE_SLOT_BUDGET_L1)
    pos_of, gid_of, gsz_of, j_of = _group_maps(groups1)
    S1 = sum(g["K"] * len(g["chunks"]) for g in groups1)

    # edge -> (core, chunk, k, lane)
    key = rank[dst]
    eorder = np.argsort(key, kind="stable")
    r_s = key[eorder]
    src_s = src[eorder]
    starts = np.searchsorted(r_s, r_s, side="left")
    k_in = np.arange(ne) - starts
    gg = r_s // P
    core_e = gg % N_CORES
    ci_e = gg // N_CORES
    lane_e = r_s % P
    ivd_e = ivd_e_all[eorder]

    # layer-1 slot columns (k-major within group)
    J1 = (np.array([g["base"] for g in groups1])[gid_of[ci_e]]
          + k_in * gsz_of[ci_e] + j_of[ci_e]) * P + lane_e

    slot1_src, slot1_sc = [], []
    node_of = []
    # storage-ordered chunk ids
    chunk_at_pos = np.empty(CPC, np.int64)
    chunk_at_pos[pos_of] = np.arange(CPC)
    for c in range(N_CORES):
        m = core_e == c
        a = np.full(S1 * P, -1, np.int64)
        a[J1[m]] = src_s[m]
        slot1_src.append(a)
        sc = np.zeros(S1 * P, np.float32)
        sc[J1[m]] = (ivd_e[m] * s1[ci_e[m]]).astype(np.float32)
        slot1_sc.append(sc)
        idx = (chunk_at_pos[:, None] * N_CORES + c) * P + np.arange(P)[None, :]
        node_of.append(nodeorder[idx.reshape(-1)].astype(np.int64))

    return dict(K=K, s1=s1, degmed=degmed, groups1=groups1, S1=S1,
                chunk_at_pos=chunk_at_pos,
                slot1_src=slot1_src, slot1_sc=slot1_sc, node_of=node_of,
                edge=dict(core=core_e, ci=ci_e, k=k_in, lane=lane_e,
                          src=src_s, ivd=ivd_e))


def _l2_layout(pp, s2):
    """Layer-2 grouping (by (ceil(K/2), s2)) + stacked-pair slot maps."""
    K2p = (pp["K"] + 1) // 2
    groups2 = _make_groups(K2p, s2, DVE_PAIR_BUDGET_L2)
    pos_of, gid_of, gsz_of, j_of = _group_maps(groups2)
    S2 = sum(g["K"] * len(g["chunks"]) for g in groups2)
    ed = pp["edge"]
    kp = ed["k"] // 2
    half = ed["k"] % 2
    J2 = (np.array([g["base"] for g in groups2])[gid_of[ed["ci"]]]
          + kp * gsz_of[ed["ci"]] + j_of[ed["ci"]]) * P + ed["lane"]
    chunk_at_pos2 = np.empty(CPC, np.int64)
    chunk_at_pos2[pos_of] = np.arange(CPC)
    node_of2 = []
    slot2_src, slot2_sc = [], []
    for c in range(N_CORES):
        m = ed["core"] == c
        at = np.full(S2 * P, -1, np.int64)
        ab = np.full(S2 * P, -1, np.int64)
        mt = m & (half == 0)
        mb = m & (half == 1)
        at[J2[mt]] = ed["src"][mt]
        ab[J2[mb]] = ed["src"][mb]
        slot2_src.append((at, ab))
        st = np.zeros(S2 * P, np.float32)
        sb = np.zeros(S2 * P, np.float32)
        st[J2[mt]] = (ed["ivd"][mt] * s2[ed["ci"][mt]]).astype(np.float32)
        sb[J2[mb]] = (ed["ivd"][mb] * s2[ed["ci"][mb]]).astype(np.float32)
        slot2_sc.append((st, sb))
        idx = (chunk_at_pos2[:, None] * N_CORES + c) * P \
            + np.arange(P)[None, :]
        # node_of2 via the same nodeorder mapping as layer 1
    # reuse layer-1 nodeorder through chunk_at_pos2
    return dict(groups2=groups2, S2=S2, chunk_at_pos2=chunk_at_pos2,
                slot2_src=slot2_src, slot2_sc=slot2_sc)


def _mk_nc():
    return bacc.Bacc(
        "TRN2",
        target_bir_lowering=False,
        debug=False,
        enable_asserts=False,
        num_devices=N_CORES,
    )


def _sections(groups, nsec):
    """Progressive sections over the slab stream, cut at group boundaries.
    Returns list of (col_a, col_b) slot-column ranges."""
    S = sum(g["K"] * len(g["chunks"]) for g in groups)
    edges = np.cumsum([0] + [g["K"] * len(g["chunks"]) for g in groups])
    base = [1.5, 1.5, 2, 3, 5] + [7] * max(nsec - 6, 0) + [4]
    fracs = np.cumsum([0] + base[:nsec])
    fracs = fracs / fracs[-1]
    cuts = [0]
    for s in range(1, nsec):
        b = int(np.searchsorted(edges, S * fracs[s]))
        cuts.append(min(max(b, cuts[-1]), len(groups)))
    cuts.append(len(groups))
    return [(int(edges[a]), int(edges[b])) for a, b in zip(cuts, cuts[1:])]


def _emit_fold(nc, sl, scr, t_ds, kg, W):
    """Emit DVE fold-in-half tree: kg group-columns of width W -> t_ds."""
    if kg == 2:
        nc.vector.tensor_tensor(out=t_ds[:, :W], in0=sl(0, 1), in1=sl(1, 2),
                                op=OP.add)
        return
    for op in _fold_schedule(kg):
        if op[0] == "L0":
            h = op[1]
            nc.vector.tensor_tensor(out=scr[:, :h * W], in0=sl(0, h),
                                    in1=sl(h, 2 * h), op=OP.add)
        elif op[0] == "odd":
            c = op[1]
            nc.vector.tensor_tensor(out=scr[:, :W], in0=scr[:, :W],
                                    in1=scr[:, (c - 1) * W:c * W], op=OP.add)
        elif op[0] == "fold":
            h = op[1]
            nc.vector.tensor_tensor(out=scr[:, :h * W], in0=scr[:, :h * W],
                                    in1=scr[:, h * W:2 * h * W], op=OP.add)
        else:
            nc.vector.tensor_tensor(out=t_ds[:, :W], in0=scr[:, :W],
                                    in1=scr[:, W:2 * W], op=OP.add)


def _flush_points(groups):
    """Output-stripe flush points: after groups nearest to 1/3, 2/3, end."""
    npos = [g["pos0"] + len(g["chunks"]) for g in groups]
    marks = []
    for frac in (0.4, 0.7, 0.9):
        tgt = int(CPC * frac)
        gi = int(np.argmin([abs(npos[i] - tgt) for i in range(len(npos))]))
        if gi not in marks:
            marks.append(gi)
    marks.append(len(groups) - 1)
    return marks


def build_layer1(pp):
    groups = pp["groups1"]
    s1 = pp["s1"]
    S1 = pp["S1"]
    secs = _sections(groups, NSEC1)
    scrw = max((g["K"] // 2 * len(g["chunks"]) for g in groups if g["dve"]),
               default=1)

    nc = _mk_nc()
    d_exp = nc.dram_tensor("expT", (P, S1 * P), E3, kind="ExternalInput")
    d_own = nc.dram_tensor("ownT", (P, NPC), BF16, kind="ExternalInput")
    d_w1 = nc.dram_tensor("w1", (C_IN, 2 * C_HID), BF16, kind="ExternalInput")
    d_w2 = nc.dram_tensor("w2", (C_HID, P), BF16, kind="ExternalInput")
    d_c1 = nc.dram_tensor("c1", (P, 1), F32, kind="ExternalInput")
    d_b2 = nc.dram_tensor("b2", (P, 1), F32, kind="ExternalInput")
    d_y2 = nc.dram_tensor("y2", (P, CPC * P), BF16, kind="ExternalOutput")

    flushes = _flush_points(groups)

    with tile.TileContext(nc) as tc:
        with (
            tc.tile_pool(name="const", bufs=1) as cp,
            tc.tile_pool(name="dsum", bufs=3) as dp,
            tc.tile_pool(name="scr", bufs=2) as sp,
            tc.tile_pool(name="psA", bufs=4, space="PSUM") as pA,
            tc.tile_pool(name="psP", bufs=2, space="PSUM") as pP,
            tc.tile_pool(name="psW", bufs=1, space="PSUM") as pW,
        ):
            t_exp = cp.tile([P, S1 * P], E3, tag="exp")
            for a, b in secs:
                if b > a:
                    nc.sync.dma_start(t_exp[:, a * P:b * P],
                                      d_exp.ap()[:, a * P:b * P])
            t_w1 = cp.tile([C_IN, 2 * C_HID], BF16, tag="w1")
            nc.scalar.dma_start(t_w1[:], d_w1.ap()[:, :])
            t_w2 = cp.tile([C_HID, P], BF16, tag="w2")
            nc.scalar.dma_start(t_w2[:], d_w2.ap()[:, :])
            t_c1 = cp.tile([P, 1], F32, tag="c1")
            nc.scalar.dma_start(t_c1[:], d_c1.ap()[:, :])
            t_b2 = cp.tile([P, 1], F32, tag="b2")
            nc.scalar.dma_start(t_b2[:], d_b2.ap()[:, :])
            t_own = cp.tile([P, NPC], BF16, tag="own")
            for a in range(0, CPC, 8):
                b = min(a + 8, CPC)
                nc.scalar.dma_start(t_own[:, a * P:b * P],
                                    d_own.ap()[:, a * P:b * P])

            t_y2all = cp.tile([P, CPC * P], BF16, tag="y2all")
            t_hall = cp.tile([P, CPC * P], BF16, tag="hall")

            t_warm = cp.tile([P, P], BF16, tag="warm")
            nc.vector.memset(t_warm[:], 1.0)
            ps_w = pW.tile([P, P], F32)
            for w in range(64):
                nc.tensor.matmul(out=ps_w[:], lhsT=t_warm[:], rhs=t_warm[:],
                                 start=(w == 0), stop=(w == 63))

            pend = None          # (ps tile is consumed by ACT; proj pipelined)
            flushed = 0
            deferred = []        # (gi, group, t_ds) folded, PE-part delayed
            done_upto = [0]      # positions with y2 complete (monotone map)

            def emit_proj(g):
                gsz = len(g["chunks"])
                W = gsz * P
                p0 = g["pos0"]
                ps2 = pP.tile([P, 4 * P], F32)
                nc.tensor.matmul(out=ps2[:, :W], lhsT=t_w2[:],
                                 rhs=t_hall[:, p0 * P:p0 * P + W],
                                 start=True, stop=True)
                nc.scalar.activation(
                    out=t_y2all[:, p0 * P:p0 * P + W],
                    in_=ps2[:, :W], func=AF.Identity,
                    bias=t_b2[:, 0:1], scale=1.0)

            def emit_pe(g):
                nonlocal pend
                gsz = len(g["chunks"])
                W = gsz * P
                kg = g["K"]
                b0 = g["base"]
                p0 = g["pos0"]
                sl = lambda j0, j1: t_exp[:, (b0 + j0 * gsz) * P:
                                          (b0 + j1 * gsz) * P]
                ps = pA.tile([P, 4 * P], F32)
                if g["dve"]:
                    nc.tensor.matmul(out=ps[:, :W], lhsT=t_w1[:, :C_HID],
                                     rhs=g["_ds"][:, :W],
                                     start=True, stop=False)
                else:
                    for k in range(kg):
                        nc.tensor.matmul(out=ps[:, :W], lhsT=t_w1[:, :C_HID],
                                         rhs=sl(k, k + 1),
                                         start=(k == 0), stop=False)
                nc.tensor.matmul(out=ps[:, :W], lhsT=t_w1[:, C_HID:],
                                 rhs=t_own[:, p0 * P:p0 * P + W],
                                 start=False, stop=True)
                nc.scalar.activation(out=t_hall[:, p0 * P:p0 * P + W],
                                     in_=ps[:, :W], func=AF.Relu,
                                     bias=t_c1[:, 0:1],
                                     scale=float(1.0 / g["s"]))
                if pend is not None:
                    emit_proj(pend)
                pend = g

            for gi, g in enumerate(groups):
                if g["dve"]:
                    gsz = len(g["chunks"])
                    W = gsz * P
                    b0 = g["base"]
                    kg = g["K"]
                    sl = lambda j0, j1: t_exp[:, (b0 + j0 * gsz) * P:
                                              (b0 + j1 * gsz) * P]
                    t_ds = dp.tile([P, 4 * P], BF16)
                    scr = sp.tile([P, scrw * P], F32)
                    _emit_fold(nc, sl, scr, t_ds, kg, W)
                    g["_ds"] = t_ds
                    deferred.append((gi, g))
                else:
                    emit_pe(g)
                while deferred and gi - deferred[0][0] >= 2:
                    emit_pe(deferred.pop(0)[1])
                if gi in flushes and flushed < CPC:
                    # stripe everything already projected (conservative:
                    # positions of groups emitted at least 2 steps back)
                    lo = min([d[1]["pos0"] for d in deferred] +
                             ([pend["pos0"]] if pend is not None else []) +
                             [CPC])
                    if lo > flushed:
                        nc.scalar.dma_start(
                            d_y2.ap()[:, flushed * P:lo * P],
                            t_y2all[:, flushed * P:lo * P])
                        flushed = lo
            for _, g in deferred:
                emit_pe(g)
            if pend is not None:
                emit_proj(pend)
            if flushed < CPC:
                nc.scalar.dma_start(
                    d_y2.ap()[:, flushed * P:CPC * P],
                    t_y2all[:, flushed * P:CPC * P])

    nc.compile()
    return nc


def build_layer2(pp, l2):
    groups = l2["groups2"]
    S2 = l2["S2"]
    secs = _sections(groups, NSEC2)
    scrw = max((g["K"] // 2 * len(g["chunks"]) for g in groups if g["dve"]),
               default=1)

    nc = _mk_nc()
    d_exp = nc.dram_tensor("expT", (P, S2 * P), E3, kind="ExternalInput")
    d_y2r = nc.dram_tensor("y2rT", (HP, NPC), BF16, kind="ExternalInput")
    d_eye = nc.dram_tensor("eye", (P, HP), BF16, kind="ExternalInput")
    d_out = nc.dram_tensor("out", (HP, CPC * P), F32, kind="ExternalOutput")

    flushes = _flush_points(groups)

    with tile.TileContext(nc) as tc:
        with (
            tc.tile_pool(name="const", bufs=1) as cp,
            tc.tile_pool(name="dsum", bufs=3) as dp,
            tc.tile_pool(name="scr", bufs=2) as sp,
            tc.tile_pool(name="psA", bufs=4, space="PSUM") as pA,
            tc.tile_pool(name="psW", bufs=1, space="PSUM") as pW,
        ):
            t_exp = cp.tile([P, S2 * P], E3, tag="exp")
            for a, b in secs:
                if b > a:
                    nc.sync.dma_start(t_exp[:, a * P:b * P],
                                      d_exp.ap()[:, a * P:b * P])
            t_eye = cp.tile([P, HP], BF16, tag="eye")
            nc.scalar.dma_start(t_eye[:], d_eye.ap()[:, :])
            t_y2r = cp.tile([HP, NPC], BF16, tag="y2r")
            for a in range(0, CPC, 8):
                b = min(a + 8, CPC)
                nc.scalar.dma_start(t_y2r[:, a * P:b * P],
                                    d_y2r.ap()[:, a * P:b * P])

            t_out = cp.tile([HP, CPC * P], F32, tag="outall")

            t_warm = cp.tile([P, P], BF16, tag="warm")
            nc.vector.memset(t_warm[:], 1.0)
            ps_w = pW.tile([P, P], F32)
            for w in range(64):
                nc.tensor.matmul(out=ps_w[:], lhsT=t_warm[:], rhs=t_warm[:],
                                 start=(w == 0), stop=(w == 63))

            flushed = 0
            deferred = []
            alt = [0]

            def emit_pe(g):
                gsz = len(g["chunks"])
                W = gsz * P
                kg = g["K"]
                b0 = g["base"]
                p0 = g["pos0"]
                sl = lambda j0, j1: t_exp[:, (b0 + j0 * gsz) * P:
                                          (b0 + j1 * gsz) * P]
                psf = pA.tile([P, 4 * P], F32)   # full bank; top half used
                ps = psf[:HP, :]
                if g["dve"]:
                    nc.tensor.matmul(out=ps[:, :W], lhsT=t_eye[:],
                                     rhs=g["_ds"][:, :W],
                                     start=True, stop=False)
                else:
                    for k in range(kg):
                        nc.tensor.matmul(out=ps[:, :W], lhsT=t_eye[:],
                                         rhs=sl(k, k + 1),
                                         start=(k == 0), stop=False)
                nc.tensor.matmul(out=ps[:, :W], lhsT=t_eye[:HP, :],
                                 rhs=t_y2r[:, p0 * P:p0 * P + W],
                                 start=False, stop=True)
                if alt[0] % 2 == 0 or g["dve"]:
                    nc.scalar.activation(out=t_out[:, p0 * P:p0 * P + W],
                                         in_=ps[:, :W], func=AF.Identity,
                                         scale=float(1.0 / g["s"]))
                else:
                    nc.vector.tensor_scalar_mul(
                        out=t_out[:, p0 * P:p0 * P + W],
                        in0=ps[:, :W], scalar1=float(1.0 / g["s"]))
                alt[0] += 1

            for gi, g in enumerate(groups):
                if g["dve"]:
                    gsz = len(g["chunks"])
                    W = gsz * P
                    b0 = g["base"]
                    kg = g["K"]
                    sl = lambda j0, j1: t_exp[:, (b0 + j0 * gsz) * P:
                                              (b0 + j1 * gsz) * P]
                    t_ds = dp.tile([P, 4 * P], BF16)
                    scr = sp.tile([P, scrw * P], F32)
                    _emit_fold(nc, sl, scr, t_ds, kg, W)
                    g["_ds"] = t_ds
                    deferred.append((gi, g))
                else:
                    emit_pe(g)
                while deferred and gi - deferred[0][0] >= 2:
                    emit_pe(deferred.pop(0)[1])
                if gi in flushes and flushed < CPC:
                    lo = min([d[1]["pos0"] for d in deferred]
                             + [g["pos0"] + len(g["chunks"])])
                    if lo > flushed:
                        nc.scalar.dma_start(
                            d_out.ap()[:, flushed * P:lo * P],
                            t_out[:, flushed * P:lo * P])
                        flushed = lo
            for _, g in deferred:
                emit_pe(g)
            if flushed < CPC:
                nc.scalar.dma_start(
                    d_out.ap()[:, flushed * P:CPC * P],
                    t_out[:, flushed * P:CPC * P])

    nc.compile()
    return nc


def _expand8(tabT_ext, slot_idx, scale, smax=15.5):
    idx = np.where(slot_idx < 0, NP_PAD, slot_idx)
    e = tabT_ext[:, idx] * scale[None, :]
    np.clip(e, -smax, smax, out=e)
    return np.ascontiguousarray(e.astype(E3_NP))


class _EmuResults:
    def __init__(self, results):
        self.results = results
        self.exec_time_ns = None
        self.mean_exec_time_ns = None
        self.max_exec_time_core_id = None


def _emu_l1(pp, m):
    expT = m["expT"].astype(np.float32)
    own = m["ownT"].astype(np.float32)
    w1 = m["w1"].astype(np.float32)
    w2 = m["w2"].astype(np.float32)
    c1 = m["c1"]; b2 = m["b2"]
    y2 = np.zeros((P, CPC * P), BF16_NP)
    hall = np.zeros((P, CPC * P), BF16_NP)
    for g in pp["groups1"]:
        gsz = len(g["chunks"]); W = gsz * P
        kg = g["K"]; b0 = g["base"]; p0 = g["pos0"]
        slabs = expT[:, b0 * P:(b0 + kg * gsz) * P].reshape(P, kg, W)
        ssum = slabs.sum(axis=1)
        if g["dve"]:
            ssum = ssum.astype(BF16_NP).astype(np.float32)
        ps = w1[:, :C_HID].T @ ssum \
            + w1[:, C_HID:].T @ own[:, p0 * P:p0 * P + W]
        h = np.maximum(ps * (1.0 / g["s"]) + c1, 0).astype(BF16_NP)
        hall[:, p0 * P:p0 * P + W] = h
        ps2 = w2.T @ h.astype(np.float32) + b2
        y2[:, p0 * P:p0 * P + W] = ps2.astype(BF16_NP)
    return {"y2": y2}


def _emu_l2(pp, l2, m):
    expT = m["expT"].astype(np.float32)
    y2r = m["y2rT"].astype(np.float32)
    out = np.zeros((HP, CPC * P), np.float32)
    for g in l2["groups2"]:
        gsz = len(g["chunks"]); W = gsz * P
        kg = g["K"]; b0 = g["base"]; p0 = g["pos0"]
        pairs = expT[:, b0 * P:(b0 + kg * gsz) * P].reshape(P, kg, W)
        psum_pair = pairs.sum(axis=1)
        if g["dve"]:
            psum_pair = psum_pair.astype(BF16_NP).astype(np.float32)
        ps = psum_pair[:HP] + psum_pair[HP:]
        ps = ps + y2r[:, p0 * P:p0 * P + W]
        out[:, p0 * P:p0 * P + W] = ps * (1.0 / g["s"])
    return {"out": out}


def kernel(x, edge_index, W1_l, W1_r, b1, bn_gamma, bn_beta, bn_mean, bn_var,
           W2_l, W2_r, b2, _results=None):
    xmax = np.zeros(NP_PAD)
    xmax[:N_NODES] = np.abs(np.asarray(x, np.float32)).max(axis=1)
    pp = _preprocess(edge_index, xmax)
    nc1 = None if _EMULATE else build_layer1(pp)

    sBN = (np.asarray(bn_gamma, np.float64)
           / np.sqrt(np.asarray(bn_var, np.float64) + BN_EPS))
    w1l_f = (np.asarray(W1_l, np.float64) * sBN[None, :]).astype(BF16_NP)
    w1r_f = (np.asarray(W1_r, np.float64) * sBN[None, :]).astype(BF16_NP)
    c1 = ((np.asarray(b1, np.float64) - np.asarray(bn_mean, np.float64)) * sBN
          + np.asarray(bn_beta, np.float64)).astype(np.float32).reshape(P, 1)
    w1 = np.ascontiguousarray(np.concatenate([w1l_f, w1r_f], axis=1))
    w2 = np.ascontiguousarray(np.concatenate(
        [np.asarray(W2_l, np.float32).astype(BF16_NP),
         np.asarray(W2_r, np.float32).astype(BF16_NP)], axis=1))
    b2col = np.concatenate([np.zeros(HP, np.float32),
                            np.asarray(b2, np.float32)]).reshape(P, 1)

    x_pad = np.zeros((NP_PAD + 1, C_IN), np.float32)
    x_pad[:N_NODES] = np.asarray(x, np.float32)
    xT_bf = np.ascontiguousarray(x_pad.astype(BF16_NP).T)
    xT_f = xT_bf.astype(np.float32)

    s1 = pp["s1"]
    own_scale = np.repeat(s1[pp["chunk_at_pos"]], P).astype(np.float32)

    maps1 = []
    for c in range(N_CORES):
        ownT = (xT_bf[:, pp["node_of"][c]].astype(np.float32)
                * own_scale[None, :]).astype(BF16_NP)
        maps1.append(dict(
            expT=_expand8(xT_f, pp["slot1_src"][c], pp["slot1_sc"][c]),
            ownT=np.ascontiguousarray(ownT),
            w1=w1, w2=w2, c1=c1, b2=b2col,
        ))
    if _EMULATE:
        r1 = _EmuResults([_emu_l1(pp, m) for m in maps1])
    else:
        r1 = run_bass_kernel_spmd(nc1, maps1, list(range(N_CORES)))

    y2lT = np.zeros((HP, NP_PAD + 1), BF16_NP)
    y2rT = np.zeros((HP, NP_PAD + 1), BF16_NP)
    for c in range(N_CORES):
        part = np.asarray(r1.results[c]["y2"])
        y2lT[:, pp["node_of"][c]] = part[:HP]
        y2rT[:, pp["node_of"][c]] = part[HP:]
    y2lT[:, NP_PAD] = 0

    # per-chunk pow2 scales for layer-2 slabs (clip-free)
    y2l_f = y2lT[:, :N_NODES].astype(np.float32)
    std = float(y2l_f.std()) + 1e-12
    y2max = np.zeros(NP_PAD)
    y2max[:N_NODES] = np.abs(y2l_f).max(axis=0)
    ed = pp["edge"]
    mx2 = np.zeros(CPC)
    np.maximum.at(mx2, ed["ci"], y2max[ed["src"]] * ed["ivd"])
    s2 = 2.0 ** np.round(np.log2(1.2 * np.maximum(pp["degmed"], 1.0) / std))
    for ci in range(CPC):
        while mx2[ci] * s2[ci] > 14.0:
            s2[ci] /= 2.0
    l2 = _l2_layout(pp, s2)
    nc2 = None if _EMULATE else build_layer2(pp, l2)

    y2l_ext = y2lT.astype(np.float32)
    eye = np.ascontiguousarray(
        np.concatenate([np.eye(HP), np.eye(HP)], axis=0).astype(BF16_NP))
    node_of2 = []
    # node_of for layer-2 storage order
    nodeorder_map = {}
    s2_at_pos2 = s2[l2["chunk_at_pos2"]]
    own2_scale = np.repeat(s2_at_pos2, P).astype(np.float32)
    maps2 = []
    for c in range(N_CORES):
        # rebuild node_of in layer-2 storage order
        no2 = pp["node_of"][c].reshape(CPC, P)
        # node_of is in layer-1 storage order; map chunk->layer2 pos
        by_chunk = np.empty((CPC, P), np.int64)
        by_chunk[pp["chunk_at_pos"]] = no2
        no2b = by_chunk[l2["chunk_at_pos2"]].reshape(-1)
        node_of2.append(no2b)
        st, sb = l2["slot2_sc"][c]
        at, ab = l2["slot2_src"][c]
        top = _expand8(y2l_ext, at, st)
        bot = _expand8(y2l_ext, ab, sb)
        expT2 = np.ascontiguousarray(np.concatenate([top, bot], axis=0))
        y2r_own = (y2rT[:, no2b].astype(np.float32)
                   * own2_scale[None, :]).astype(BF16_NP)
        maps2.append(dict(
            expT=expT2, y2rT=np.ascontiguousarray(y2r_own), eye=eye,
        ))
    if _EMULATE:
        r2 = _EmuResults([_emu_l2(pp, l2, m) for m in maps2])
    else:
        r2 = run_bass_kernel_spmd(nc2, maps2, list(range(N_CORES)))

    out = np.zeros((NP_PAD, C_OUT), np.float32)
    for c in range(N_CORES):
        part = np.asarray(r2.results[c]["out"])
        out[node_of2[c]] = part.T
    if _results is not None:
        _results.extend([r1, r2])
    return np.ascontiguousarray(out[:N_NODES])


# revision 30
# speedup vs baseline: 1.6134x; 1.0828x over previous
"""2-layer GraphSAGE (mean aggr + BN(eval) + ReLU) on Trainium2, 8-core SPMD.

Strategy (dst-node sharding, host-mediated all-to-all, fp8 slabs, grouped
full-bank psum pipeline):
  - Host: relabel nodes by in-degree (desc), deal 128-node chunks round-robin
    to 8 cores (chunk ci has ~equal degrees on every core -> shared pad depth
    K[ci], SPMD). Consecutive chunks with equal (K, scale) form GROUPS of up
    to 4; each group owns a full PSUM bank [128, gsz*128] so the
    PE->ACT->PE pipeline never shares banks (per-chunk psum tiles caused
    bank-conflict serialization at ~1.1us/chunk).
  - Slabs are fp8-e3m4, pre-scaled by invdeg * 2^s(ci) (per-chunk pow2,
    capped so nothing clips; inverse applied by ACT at psum readout). Slot
    layout is k-major within a group, so ONE matmul per k covers the whole
    group (moving [128ch, gsz*128]).
  - Layer 1: W1 rides STATIONARY in the PE; slabs stream as moving operand.
    High-K groups are pre-reduced on the Vector engine (fold-in-half tree,
    f32 scratch, bf16 final). Per group:
       psum[chout, g*dst] = sum_k W1l^T slab_k (+ W1l^T dve_sum)
                          + W1r^T own          (own = x*2^s, bf16)
       h  = ACT(Relu, scale=2^-s, bias=c1)     (BN folded into W1/c1)
       psum2 = [W2l|W2r]^T h                   (one fused projection matmul)
       y2 = psum2 + [0;b2]                     (GPSIMD tensor_scalar_add)
    Only y2 ([y2l;y2r], bf16) returns to the host - h never does.
  - Host: regather of y2l into layer-2 slabs: fp8 stacked PAIRS ([2x64ch])
    pre-scaled by invdeg * 2^s2(ci); y2r (own dst, includes b2) stays bf16,
    pre-scaled by 2^s2(ci).
  - Layer 2: aggregation is a pure sum: stacked pairs contract with a
    constant [I64;I64] stationary; DVE pre-folds high-K groups; y2r joins
    via an I64 matmul; ACT scales by 2^-s2 to f32 out. No weights on device.
"""

import os

import numpy as np

import concourse.bacc as bacc
import concourse.mybir as mybir
import concourse.tile as tile
from concourse.bass_utils import run_bass_kernel_spmd

F32 = mybir.dt.float32
BF16 = mybir.dt.bfloat16
E3 = mybir.dt.float8e3
OP = mybir.AluOpType
AF = mybir.ActivationFunctionType
BF16_NP = mybir.dt.np(mybir.dt.bfloat16)
E3_NP = mybir.dt.np(mybir.dt.float8e3)

N_CORES = 8
P = 128
HP = 64

N_NODES = 50000
NP_PAD = 50176            # 392 chunks of 128
C_IN, C_HID, C_OUT = 128, 128, 64
CPC = NP_PAD // P // N_CORES   # 49 chunks per core
NPC = CPC * P                  # 6272 nodes per core
BN_EPS = 1e-5

# tuning knobs
DVE_SLOT_BUDGET_L1 = 160       # ~ slots pre-reduced on DVE in layer 1
DVE_PAIR_BUDGET_L2 = 70        # ~ pair-columns pre-reduced on DVE in layer 2
GROUP_MAX = 4
NSEC1 = 10
NSEC2 = 8
_EMULATE = bool(os.environ.get("KERNEL_EMULATE"))


def _fold_schedule(m):
    """Fold-in-half schedule for m group-columns -> 2 (then a final add).

    ('L0', h): scr[0:h] = in[0:h] + in[h:2h]     (m even, h=m//2)
    ('odd', c): scr[0] += scr[c-1]
    ('fold', h): scr[0:h] += scr[h:2h]
    ('final',): out = scr[0] + scr[1]            (bf16)
    """
    assert m % 2 == 0 and m >= 4
    ops = [("L0", m // 2)]
    m //= 2
    while m > 2:
        if m % 2 == 1:
            ops.append(("odd", m))
            m -= 1
        if m == 2:
            break
        ops.append(("fold", m // 2))
        m //= 2
    ops.append(("final",))
    return ops


def _make_groups(Kv, sv, budget, min_fold=2):
    """Group consecutive chunks (K-desc chunk ids) with equal (K, scale),
    size<=GROUP_MAX; mark top-K groups as DVE until slot budget is used;
    interleave DVE among PE groups for engine overlap.

    Returns list of dicts: chunks, K (padded even for DVE), s, dve, plus
    storage fields pos0/base filled later.
    """
    groups = []
    i = 0
    while i < CPC:
        j = i
        while (j < CPC and j - i < GROUP_MAX and Kv[j] == Kv[i]
               and sv[j] == sv[i]):
            j += 1
        groups.append(dict(chunks=list(range(i, j)), K=int(Kv[i]),
                           s=float(sv[i]), dve=False))
        i = j
    tot = 0
    for g in groups:                       # groups are K-desc already
        cost = g["K"] * len(g["chunks"])
        if tot + cost <= budget and g["K"] >= min_fold:
            g["dve"] = True
            tot += cost
            if g["K"] % 2 and g["K"] > 1:
                g["K"] += 1                # even K for clean folds
    dlist = [g for g in groups if g["dve"]]
    plist = [g for g in groups if not g["dve"]]
    out = []
    di = pi = 0
    ratio = max(len(plist) / max(len(dlist), 1), 1.0)
    while di < len(dlist) or pi < len(plist):
        if di < len(dlist) and (pi >= len(plist) or pi >= ratio * di):
            out.append(dlist[di]); di += 1
        else:
            out.append(plist[pi]); pi += 1
    pos = 0
    base = 0
    for g in out:
        g["pos0"] = pos
        g["base"] = base
        pos += len(g["chunks"])
        base += g["K"] * len(g["chunks"])
    return out


def _group_maps(groups):
    """Per-chunk lookup arrays: storage pos, group id."""
    pos_of = np.empty(CPC, np.int64)
    gid_of = np.empty(CPC, np.int64)
    gsz_of = np.empty(CPC, np.int64)
    j_of = np.empty(CPC, np.int64)
    for gi, g in enumerate(groups):
        for jj, ci in enumerate(g["chunks"]):
            pos_of[ci] = g["pos0"] + jj
            gid_of[ci] = gi
            gsz_of[ci] = len(g["chunks"])
            j_of[ci] = jj
    return pos_of, gid_of, gsz_of, j_of


def _preprocess(edge_index, xmax):
    """Degree-sort relabeling, layer-1 grouping/slot maps, edge metadata."""
    src = np.asarray(edge_index[0]).astype(np.int64)
    dst = np.asarray(edge_index[1]).astype(np.int64)
    ne = src.shape[0]
    deg = np.bincount(dst, minlength=NP_PAD).astype(np.int64)

    nodeorder = np.argsort(-deg, kind="stable")        # rank -> node
    rank = np.empty(NP_PAD, np.int64)
    rank[nodeorder] = np.arange(NP_PAD)

    gdeg3 = deg[nodeorder].reshape(CPC, N_CORES, P)
    K = np.maximum(gdeg3.max(axis=(1, 2)), 1)
    degmed = np.maximum(np.median(gdeg3.reshape(CPC, -1), axis=1), 1.0)
    s1 = 2.0 ** np.round(np.log2(2.0 * degmed))
    # cap so no slab value exceeds e3m4 range
    ci_of_all = rank[dst] // P // N_CORES
    ivd_e_all = 1.0 / np.maximum(deg[dst], 1.0)
    mx1 = np.zeros(CPC)
    np.maximum.at(mx1, ci_of_all, np.asarray(xmax)[src] * ivd_e_all)
    for ci in range(CPC):
        while mx1[ci] * s1[ci] > 14.0:
            s1[ci] /= 2.0

    groups1 = _make_groups(K, s1, DVE_SLOT_BUDGET_L1)
    pos_of, gid_of, gsz_of, j_of = _group_maps(groups1)
    S1 = sum(g["K"] * len(g["chunks"]) for g in groups1)

    # edge -> (core, chunk, k, lane)
    key = rank[dst]
    eorder = np.argsort(key, kind="stable")
    r_s = key[eorder]
    src_s = src[eorder]
    starts = np.searchsorted(r_s, r_s, side="left")
    k_in = np.arange(ne) - starts
    gg = r_s // P
    core_e = gg % N_CORES
    ci_e = gg // N_CORES
    lane_e = r_s % P
    ivd_e = ivd_e_all[eorder]

    # layer-1 slot columns (k-major within group)
    J1 = (np.array([g["base"] for g in groups1])[gid_of[ci_e]]
          + k_in * gsz_of[ci_e] + j_of[ci_e]) * P + lane_e

    slot1_src, slot1_sc = [], []
    node_of = []
    # storage-ordered chunk ids
    chunk_at_pos = np.empty(CPC, np.int64)
    chunk_at_pos[pos_of] = np.arange(CPC)
    for c in range(N_CORES):
        m = core_e == c
        a = np.full(S1 * P, -1, np.int64)
        a[J1[m]] = src_s[m]
        slot1_src.append(a)
        sc = np.zeros(S1 * P, np.float32)
        sc[J1[m]] = (ivd_e[m] * s1[ci_e[m]]).astype(np.float32)
        slot1_sc.append(sc)
        idx = (chunk_at_pos[:, None] * N_CORES + c) * P + np.arange(P)[None, :]
        node_of.append(nodeorder[idx.reshape(-1)].astype(np.int64))

    return dict(K=K, s1=s1, degmed=degmed, groups1=groups1, S1=S1,
                chunk_at_pos=chunk_at_pos,
                slot1_src=slot1_src, slot1_sc=slot1_sc, node_of=node_of,
                edge=dict(core=core_e, ci=ci_e, k=k_in, lane=lane_e,
                          src=src_s, ivd=ivd_e))


def _l2_layout(pp, s2):
    """Layer-2 grouping (by (ceil(K/2), s2)) + stacked-pair slot maps."""
    K2p = (pp["K"] + 1) // 2
    groups2 = _make_groups(K2p, s2, DVE_PAIR_BUDGET_L2)
    pos_of, gid_of, gsz_of, j_of = _group_maps(groups2)
    S2 = sum(g["K"] * len(g["chunks"]) for g in groups2)
    ed = pp["edge"]
    kp = ed["k"] // 2
    half = ed["k"] % 2
    J2 = (np.array([g["base"] for g in groups2])[gid_of[ed["ci"]]]
          + kp * gsz_of[ed["ci"]] + j_of[ed["ci"]]) * P + ed["lane"]
    chunk_at_pos2 = np.empty(CPC, np.int64)
    chunk_at_pos2[pos_of] = np.arange(CPC)
    node_of2 = []
    slot2_src, slot2_sc = [], []
    for c in range(N_CORES):
        m = ed["core"] == c
        at = np.full(S2 * P, -1, np.int64)
        ab = np.full(S2 * P, -1, np.int64)
        mt = m & (half == 0)
        mb = m & (half == 1)
        at[J2[mt]] = ed["src"][mt]
        ab[J2[mb]] = ed["src"][mb]
        slot2_src.append((at, ab))
        st = np.zeros(S2 * P, np.float32)
        sb = np.zeros(S2 * P, np.float32)
        st[J2[mt]] = (ed["ivd"][mt] * s2[ed["ci"][mt]]).astype(np.float32)
        sb[J2[mb]] = (ed["ivd"][mb] * s2[ed["ci"][mb]]).astype(np.float32)
        slot2_sc.append((st, sb))
        idx = (chunk_at_pos2[:, None] * N_CORES + c) * P \
            + np.arange(P)[None, :]
        # node_of2 via the same nodeorder mapping as layer 1
    # reuse layer-1 nodeorder through chunk_at_pos2
    return dict(groups2=groups2, S2=S2, chunk_at_pos2=chunk_at_pos2,
                slot2_src=slot2_src, slot2_sc=slot2_sc)


def _mk_nc():
    return bacc.Bacc(
        "TRN2",
        target_bir_lowering=False,
        debug=False,
        enable_asserts=False,
        num_devices=N_CORES,
    )


def _sections(groups, nsec):
    """Progressive sections over the slab stream, cut at group boundaries.
    Returns list of (col_a, col_b) slot-column ranges."""
    S = sum(g["K"] * len(g["chunks"]) for g in groups)
    edges = np.cumsum([0] + [g["K"] * len(g["chunks"]) for g in groups])
    base = [1.5, 2, 2.5] + [3] * max(nsec - 3, 0)
    fracs = np.cumsum([0] + base[:nsec])
    fracs = fracs / fracs[-1]
    cuts = [0]
    for s in range(1, nsec):
        b = int(np.searchsorted(edges, S * fracs[s]))
        cuts.append(min(max(b, cuts[-1]), len(groups)))
    cuts.append(len(groups))
    return [(int(edges[a]), int(edges[b])) for a, b in zip(cuts, cuts[1:])]


def _emit_fold(nc, sl, scr, t_ds, kg, W):
    """Emit DVE fold-in-half tree: kg group-columns of width W -> t_ds."""
    if kg == 2:
        nc.vector.tensor_tensor(out=t_ds[:, :W], in0=sl(0, 1), in1=sl(1, 2),
                                op=OP.add)
        return
    for op in _fold_schedule(kg):
        if op[0] == "L0":
            h = op[1]
            nc.vector.tensor_tensor(out=scr[:, :h * W], in0=sl(0, h),
                                    in1=sl(h, 2 * h), op=OP.add)
        elif op[0] == "odd":
            c = op[1]
            nc.vector.tensor_tensor(out=scr[:, :W], in0=scr[:, :W],
                                    in1=scr[:, (c - 1) * W:c * W], op=OP.add)
        elif op[0] == "fold":
            h = op[1]
            nc.vector.tensor_tensor(out=scr[:, :h * W], in0=scr[:, :h * W],
                                    in1=scr[:, h * W:2 * h * W], op=OP.add)
        else:
            nc.vector.tensor_tensor(out=t_ds[:, :W], in0=scr[:, :W],
                                    in1=scr[:, W:2 * W], op=OP.add)


def _flush_points(groups):
    """Output-stripe flush points: after groups nearest to 1/3, 2/3, end."""
    npos = [g["pos0"] + len(g["chunks"]) for g in groups]
    marks = []
    for frac in (0.22, 0.38, 0.52, 0.65, 0.76, 0.86, 0.94):
        tgt = int(CPC * frac)
        gi = int(np.argmin([abs(npos[i] - tgt) for i in range(len(npos))]))
        if gi not in marks:
            marks.append(gi)
    marks.append(len(groups) - 1)
    return marks


def build_layer1(pp):
    groups = pp["groups1"]
    s1 = pp["s1"]
    S1 = pp["S1"]
    secs = _sections(groups, NSEC1)
    scrw = max((g["K"] // 2 * len(g["chunks"]) for g in groups if g["dve"]),
               default=1)

    nc = _mk_nc()
    d_exp = nc.dram_tensor("expT", (P, S1 * P), E3, kind="ExternalInput")
    d_own = nc.dram_tensor("ownT", (P, NPC), BF16, kind="ExternalInput")
    d_w1 = nc.dram_tensor("w1", (C_IN, 2 * C_HID), BF16, kind="ExternalInput")
    d_w2 = nc.dram_tensor("w2", (C_HID, P), BF16, kind="ExternalInput")
    d_c1 = nc.dram_tensor("c1", (P, 1), F32, kind="ExternalInput")
    d_b2 = nc.dram_tensor("b2", (P, 1), F32, kind="ExternalInput")
    d_y2 = nc.dram_tensor("y2", (P, CPC * P), BF16, kind="ExternalOutput")

    flushes = _flush_points(groups)

    with tile.TileContext(nc) as tc:
        with (
            tc.tile_pool(name="const", bufs=1) as cp,
            tc.tile_pool(name="dsum", bufs=3) as dp,
            tc.tile_pool(name="scr", bufs=2) as sp,
            tc.tile_pool(name="psA", bufs=4, space="PSUM") as pA,
            tc.tile_pool(name="psP", bufs=2, space="PSUM") as pP,
            tc.tile_pool(name="psW", bufs=1, space="PSUM") as pW,
        ):
            t_exp = cp.tile([P, S1 * P], E3, tag="exp")
            for a, b in secs:
                if b > a:
                    nc.sync.dma_start(t_exp[:, a * P:b * P],
                                      d_exp.ap()[:, a * P:b * P])
            t_w1 = cp.tile([C_IN, 2 * C_HID], BF16, tag="w1")
            nc.scalar.dma_start(t_w1[:], d_w1.ap()[:, :])
            t_w2 = cp.tile([C_HID, P], BF16, tag="w2")
            nc.scalar.dma_start(t_w2[:], d_w2.ap()[:, :])
            t_c1 = cp.tile([P, 1], F32, tag="c1")
            nc.scalar.dma_start(t_c1[:], d_c1.ap()[:, :])
            t_b2 = cp.tile([P, 1], F32, tag="b2")
            nc.scalar.dma_start(t_b2[:], d_b2.ap()[:, :])
            t_own = cp.tile([P, NPC], BF16, tag="own")
            for a in range(0, CPC, 8):
                b = min(a + 8, CPC)
                nc.scalar.dma_start(t_own[:, a * P:b * P],
                                    d_own.ap()[:, a * P:b * P])

            t_y2all = cp.tile([P, CPC * P], BF16, tag="y2all")
            t_hall = cp.tile([P, CPC * P], BF16, tag="hall")

            t_warm = cp.tile([P, P], BF16, tag="warm")
            nc.vector.memset(t_warm[:], 1.0)
            ps_w = pW.tile([P, P], F32)
            for w in range(64):
                nc.tensor.matmul(out=ps_w[:], lhsT=t_warm[:], rhs=t_warm[:],
                                 start=(w == 0), stop=(w == 63))

            pend = None          # (ps tile is consumed by ACT; proj pipelined)
            flushed = 0
            deferred = []        # (gi, group, t_ds) folded, PE-part delayed
            done_upto = [0]      # positions with y2 complete (monotone map)

            def emit_proj(g):
                gsz = len(g["chunks"])
                W = gsz * P
                p0 = g["pos0"]
                ps2 = pP.tile([P, 4 * P], F32)
                nc.tensor.matmul(out=ps2[:, :W], lhsT=t_w2[:],
                                 rhs=t_hall[:, p0 * P:p0 * P + W],
                                 start=True, stop=True)
                nc.scalar.activation(
                    out=t_y2all[:, p0 * P:p0 * P + W],
                    in_=ps2[:, :W], func=AF.Identity,
                    bias=t_b2[:, 0:1], scale=1.0)

            def emit_pe(g):
                nonlocal pend
                gsz = len(g["chunks"])
                W = gsz * P
                kg = g["K"]
                b0 = g["base"]
                p0 = g["pos0"]
                sl = lambda j0, j1: t_exp[:, (b0 + j0 * gsz) * P:
                                          (b0 + j1 * gsz) * P]
                ps = pA.tile([P, 4 * P], F32)
                if g["dve"]:
                    nc.tensor.matmul(out=ps[:, :W], lhsT=t_w1[:, :C_HID],
                                     rhs=g["_ds"][:, :W],
                                     start=True, stop=False)
                else:
                    for k in range(kg):
                        nc.tensor.matmul(out=ps[:, :W], lhsT=t_w1[:, :C_HID],
                                         rhs=sl(k, k + 1),
                                         start=(k == 0), stop=False)
                nc.tensor.matmul(out=ps[:, :W], lhsT=t_w1[:, C_HID:],
                                 rhs=t_own[:, p0 * P:p0 * P + W],
                                 start=False, stop=True)
                nc.scalar.activation(out=t_hall[:, p0 * P:p0 * P + W],
                                     in_=ps[:, :W], func=AF.Relu,
                                     bias=t_c1[:, 0:1],
                                     scale=float(1.0 / g["s"]))
                if pend is not None:
                    emit_proj(pend)
                pend = g

            for gi, g in enumerate(groups):
                if g["dve"]:
                    gsz = len(g["chunks"])
                    W = gsz * P
                    b0 = g["base"]
                    kg = g["K"]
                    sl = lambda j0, j1: t_exp[:, (b0 + j0 * gsz) * P:
                                              (b0 + j1 * gsz) * P]
                    t_ds = dp.tile([P, 4 * P], BF16)
                    scr = sp.tile([P, scrw * P], F32)
                    _emit_fold(nc, sl, scr, t_ds, kg, W)
                    g["_ds"] = t_ds
                    deferred.append((gi, g))
                else:
                    emit_pe(g)
                while deferred and gi - deferred[0][0] >= 2:
                    emit_pe(deferred.pop(0)[1])
                if gi in flushes and flushed < CPC:
                    # stripe everything already projected (conservative:
                    # positions of groups emitted at least 2 steps back)
                    lo = min([d[1]["pos0"] for d in deferred] +
                             ([pend["pos0"]] if pend is not None else []) +
                             [CPC])
                    if lo > flushed:
                        nc.scalar.dma_start(
                            d_y2.ap()[:, flushed * P:lo * P],
                            t_y2all[:, flushed * P:lo * P])
                        flushed = lo
            for _, g in deferred:
                emit_pe(g)
            if pend is not None:
                emit_proj(pend)
            if flushed < CPC:
                nc.scalar.dma_start(
                    d_y2.ap()[:, flushed * P:CPC * P],
                    t_y2all[:, flushed * P:CPC * P])

    nc.compile()
    return nc


def build_layer2(pp, l2):
    groups = l2["groups2"]
    S2 = l2["S2"]
    secs = _sections(groups, NSEC2)
    scrw = max((g["K"] // 2 * len(g["chunks"]) for g in groups if g["dve"]),
               default=1)

    nc = _mk_nc()
    d_exp = nc.dram_tensor("expT", (P, S2 * P), E3, kind="ExternalInput")
    d_y2r = nc.dram_tensor("y2rT", (HP, NPC), BF16, kind="ExternalInput")
    d_eye = nc.dram_tensor("eye", (P, HP), BF16, kind="ExternalInput")
    d_out = nc.dram_tensor("out", (HP, CPC * P), F32, kind="ExternalOutput")

    flushes = _flush_points(groups)

    with tile.TileContext(nc) as tc:
        with (
            tc.tile_pool(name="const", bufs=1) as cp,
            tc.tile_pool(name="dsum", bufs=3) as dp,
            tc.tile_pool(name="scr", bufs=2) as sp,
            tc.tile_pool(name="psA", bufs=4, space="PSUM") as pA,
            tc.tile_pool(name="psW", bufs=1, space="PSUM") as pW,
        ):
            t_exp = cp.tile([P, S2 * P], E3, tag="exp")
            for a, b in secs:
                if b > a:
                    nc.sync.dma_start(t_exp[:, a * P:b * P],
                                      d_exp.ap()[:, a * P:b * P])
            t_eye = cp.tile([P, HP], BF16, tag="eye")
            nc.scalar.dma_start(t_eye[:], d_eye.ap()[:, :])
            t_y2r = cp.tile([HP, NPC], BF16, tag="y2r")
            for a in range(0, CPC, 8):
                b = min(a + 8, CPC)
                nc.scalar.dma_start(t_y2r[:, a * P:b * P],
                                    d_y2r.ap()[:, a * P:b * P])

            t_out = cp.tile([HP, CPC * P], F32, tag="outall")

            t_warm = cp.tile([P, P], BF16, tag="warm")
            nc.vector.memset(t_warm[:], 1.0)
            ps_w = pW.tile([P, P], F32)
            for w in range(64):
                nc.tensor.matmul(out=ps_w[:], lhsT=t_warm[:], rhs=t_warm[:],
                                 start=(w == 0), stop=(w == 63))

            flushed = 0
            deferred = []
            alt = [0]

            def emit_pe(g):
                gsz = len(g["chunks"])
                W = gsz * P
                kg = g["K"]
                b0 = g["base"]
                p0 = g["pos0"]
                sl = lambda j0, j1: t_exp[:, (b0 + j0 * gsz) * P:
                                          (b0 + j1 * gsz) * P]
                psf = pA.tile([P, 4 * P], F32)   # full bank; top half used
                ps = psf[:HP, :]
                if g["dve"]:
                    nc.tensor.matmul(out=ps[:, :W], lhsT=t_eye[:],
                                     rhs=g["_ds"][:, :W],
                                     start=True, stop=False)
                else:
                    for k in range(kg):
                        nc.tensor.matmul(out=ps[:, :W], lhsT=t_eye[:],
                                         rhs=sl(k, k + 1),
                                         start=(k == 0), stop=False)
                nc.tensor.matmul(out=ps[:, :W], lhsT=t_eye[:HP, :],
                                 rhs=t_y2r[:, p0 * P:p0 * P + W],
                                 start=False, stop=True)
                if alt[0] % 2 == 0 or g["dve"]:
                    nc.scalar.activation(out=t_out[:, p0 * P:p0 * P + W],
                                         in_=ps[:, :W], func=AF.Identity,
                                         scale=float(1.0 / g["s"]))
                else:
                    nc.vector.tensor_scalar_mul(
                        out=t_out[:, p0 * P:p0 * P + W],
                        in0=ps[:, :W], scalar1=float(1.0 / g["s"]))
                alt[0] += 1

            for gi, g in enumerate(groups):
                if g["dve"]:
                    gsz = len(g["chunks"])
                    W = gsz * P
                    b0 = g["base"]
                    kg = g["K"]
                    sl = lambda j0, j1: t_exp[:, (b0 + j0 * gsz) * P:
                                              (b0 + j1 * gsz) * P]
                    t_ds = dp.tile([P, 4 * P], BF16)
                    scr = sp.tile([P, scrw * P], F32)
                    _emit_fold(nc, sl, scr, t_ds, kg, W)
                    g["_ds"] = t_ds
                    deferred.append((gi, g))
                else:
                    emit_pe(g)
                while deferred and gi - deferred[0][0] >= 2:
                    emit_pe(deferred.pop(0)[1])
                if gi in flushes and flushed < CPC:
                    lo = min([d[1]["pos0"] for d in deferred]
                             + [g["pos0"] + len(g["chunks"])])
                    if lo > flushed:
                        nc.scalar.dma_start(
                            d_out.ap()[:, flushed * P:lo * P],
                            t_out[:, flushed * P:lo * P])
                        flushed = lo
            for _, g in deferred:
                emit_pe(g)
            if flushed < CPC:
                nc.scalar.dma_start(
                    d_out.ap()[:, flushed * P:CPC * P],
                    t_out[:, flushed * P:CPC * P])

    nc.compile()
    return nc


def _expand8(tabT_ext, slot_idx, scale, smax=15.5):
    idx = np.where(slot_idx < 0, NP_PAD, slot_idx)
    e = tabT_ext[:, idx] * scale[None, :]
    np.clip(e, -smax, smax, out=e)
    return np.ascontiguousarray(e.astype(E3_NP))


class _EmuResults:
    def __init__(self, results):
        self.results = results
        self.exec_time_ns = None
        self.mean_exec_time_ns = None
        self.max_exec_time_core_id = None


def _emu_l1(pp, m):
    expT = m["expT"].astype(np.float32)
    own = m["ownT"].astype(np.float32)
    w1 = m["w1"].astype(np.float32)
    w2 = m["w2"].astype(np.float32)
    c1 = m["c1"]; b2 = m["b2"]
    y2 = np.zeros((P, CPC * P), BF16_NP)
    hall = np.zeros((P, CPC * P), BF16_NP)
    for g in pp["groups1"]:
        gsz = len(g["chunks"]); W = gsz * P
        kg = g["K"]; b0 = g["base"]; p0 = g["pos0"]
        slabs = expT[:, b0 * P:(b0 + kg * gsz) * P].reshape(P, kg, W)
        ssum = slabs.sum(axis=1)
        if g["dve"]:
            ssum = ssum.astype(BF16_NP).astype(np.float32)
        ps = w1[:, :C_HID].T @ ssum \
            + w1[:, C_HID:].T @ own[:, p0 * P:p0 * P + W]
        h = np.maximum(ps * (1.0 / g["s"]) + c1, 0).astype(BF16_NP)
        hall[:, p0 * P:p0 * P + W] = h
        ps2 = w2.T @ h.astype(np.float32) + b2
        y2[:, p0 * P:p0 * P + W] = ps2.astype(BF16_NP)
    return {"y2": y2}


def _emu_l2(pp, l2, m):
    expT = m["expT"].astype(np.float32)
    y2r = m["y2rT"].astype(np.float32)
    out = np.zeros((HP, CPC * P), np.float32)
    for g in l2["groups2"]:
        gsz = len(g["chunks"]); W = gsz * P
        kg = g["K"]; b0 = g["base"]; p0 = g["pos0"]
        pairs = expT[:, b0 * P:(b0 + kg * gsz) * P].reshape(P, kg, W)
        psum_pair = pairs.sum(axis=1)
        if g["dve"]:
            psum_pair = psum_pair.astype(BF16_NP).astype(np.float32)
        ps = psum_pair[:HP] + psum_pair[HP:]
        ps = ps + y2r[:, p0 * P:p0 * P + W]
        out[:, p0 * P:p0 * P + W] = ps * (1.0 / g["s"])
    return {"out": out}


def kernel(x, edge_index, W1_l, W1_r, b1, bn_gamma, bn_beta, bn_mean, bn_var,
           W2_l, W2_r, b2, _results=None):
    xmax = np.zeros(NP_PAD)
    xmax[:N_NODES] = np.abs(np.asarray(x, np.float32)).max(axis=1)
    pp = _preprocess(edge_index, xmax)
    nc1 = None if _EMULATE else build_layer1(pp)

    sBN = (np.asarray(bn_gamma, np.float64)
           / np.sqrt(np.asarray(bn_var, np.float64) + BN_EPS))
    w1l_f = (np.asarray(W1_l, np.float64) * sBN[None, :]).astype(BF16_NP)
    w1r_f = (np.asarray(W1_r, np.float64) * sBN[None, :]).astype(BF16_NP)
    c1 = ((np.asarray(b1, np.float64) - np.asarray(bn_mean, np.float64)) * sBN
          + np.asarray(bn_beta, np.float64)).astype(np.float32).reshape(P, 1)
    w1 = np.ascontiguousarray(np.concatenate([w1l_f, w1r_f], axis=1))
    w2 = np.ascontiguousarray(np.concatenate(
        [np.asarray(W2_l, np.float32).astype(BF16_NP),
         np.asarray(W2_r, np.float32).astype(BF16_NP)], axis=1))
    b2col = np.concatenate([np.zeros(HP, np.float32),
                            np.asarray(b2, np.float32)]).reshape(P, 1)

    x_pad = np.zeros((NP_PAD + 1, C_IN), np.float32)
    x_pad[:N_NODES] = np.asarray(x, np.float32)
    xT_bf = np.ascontiguousarray(x_pad.astype(BF16_NP).T)
    xT_f = xT_bf.astype(np.float32)

    s1 = pp["s1"]
    own_scale = np.repeat(s1[pp["chunk_at_pos"]], P).astype(np.float32)

    maps1 = []
    for c in range(N_CORES):
        ownT = (xT_bf[:, pp["node_of"][c]].astype(np.float32)
                * own_scale[None, :]).astype(BF16_NP)
        maps1.append(dict(
            expT=_expand8(xT_f, pp["slot1_src"][c], pp["slot1_sc"][c]),
            ownT=np.ascontiguousarray(ownT),
            w1=w1, w2=w2, c1=c1, b2=b2col,
        ))
    if _EMULATE:
        r1 = _EmuResults([_emu_l1(pp, m) for m in maps1])
    else:
        r1 = run_bass_kernel_spmd(nc1, maps1, list(range(N_CORES)))

    y2lT = np.zeros((HP, NP_PAD + 1), BF16_NP)
    y2rT = np.zeros((HP, NP_PAD + 1), BF16_NP)
    for c in range(N_CORES):
        part = np.asarray(r1.results[c]["y2"])
        y2lT[:, pp["node_of"][c]] = part[:HP]
        y2rT[:, pp["node_of"][c]] = part[HP:]
    y2lT[:, NP_PAD] = 0

    # per-chunk pow2 scales for layer-2 slabs (clip-free)
    y2l_f = y2lT[:, :N_NODES].astype(np.float32)
    std = float(y2l_f.std()) + 1e-12
    y2max = np.zeros(NP_PAD)
    y2max[:N_NODES] = np.abs(y2l_f).max(axis=0)
    ed = pp["edge"]
    mx2 = np.zeros(CPC)
    np.maximum.at(mx2, ed["ci"], y2max[ed["src"]] * ed["ivd"])
    s2 = 2.0 ** np.round(np.log2(1.2 * np.maximum(pp["degmed"], 1.0) / std))
    for ci in range(CPC):
        while mx2[ci] * s2[ci] > 14.0:
            s2[ci] /= 2.0
    l2 = _l2_layout(pp, s2)
    nc2 = None if _EMULATE else build_layer2(pp, l2)

    y2l_ext = y2lT.astype(np.float32)
    eye = np.ascontiguousarray(
        np.concatenate([np.eye(HP), np.eye(HP)], axis=0).astype(BF16_NP))
    node_of2 = []
    # node_of for layer-2 storage order
    nodeorder_map = {}
    s2_at_pos2 = s2[l2["chunk_at_pos2"]]
    own2_scale = np.repeat(s2_at_pos2, P).astype(np.float32)
    maps2 = []
    for c in range(N_CORES):
        # rebuild node_of in layer-2 storage order
        no2 = pp["node_of"][c].reshape(CPC, P)
        # node_of is in layer-1 storage order; map chunk->layer2 pos
        by_chunk = np.empty((CPC, P), np.int64)
        by_chunk[pp["chunk_at_pos"]] = no2
        no2b = by_chunk[l2["chunk_at_pos2"]].reshape(-1)
        node_of2.append(no2b)
        st, sb = l2["slot2_sc"][c]
        at, ab = l2["slot2_src"][c]
        top = _expand8(y2l_ext, at, st)
        bot = _expand8(y2l_ext, ab, sb)
        expT2 = np.ascontiguousarray(np.concatenate([top, bot], axis=0))
        y2r_own = (y2rT[:, no2b].astype(np.float32)
                   * own2_scale[None, :]).astype(BF16_NP)
        maps2.append(dict(
            expT=expT2, y2rT=np.ascontiguousarray(y2r_own), eye=eye,
        ))
    if _EMULATE:
        r2 = _EmuResults([_emu_l2(pp, l2, m) for m in maps2])
    else:
        r2 = run_bass_kernel_spmd(nc2, maps2, list(range(N_CORES)))

    out = np.zeros((NP_PAD, C_OUT), np.float32)
    for c in range(N_CORES):
        part = np.asarray(r2.results[c]["out"])
        out[node_of2[c]] = part.T
    if _results is not None:
        _results.extend([r1, r2])
    return np.ascontiguousarray(out[:N_NODES])
